# revision 5
# baseline (speedup 1.0000x reference)
"""AttnDecoderRNN single-step decoder on 8 Trainium2 NeuronCores.

Tensor-parallel sharding (hardcoded, 8 cores):
  - GRU layer 0: hidden units sharded (128 per core), full contraction.
  - GRU layer 1: contraction sharded; partial (gi1|gh1|h0) fused AllReduce.
  - Attention: W_attn/encoder columns sharded; partial-energy AllReduce;
    softmax replicated; context column-sharded (no collective).
  - Output projection: contraction (2H) sharded 256/core; partial-logits
    AllReduce; log-softmax replicated locally.
Embedding row gather happens on host (only one row of emb is ever read).
"""

import numpy as np

import concourse.bacc as bacc
import concourse.bass as bass
import concourse.mybir as mybir
import concourse.tile as tile
from concourse import bass_isa
from concourse import library_config
from concourse.bass_utils import run_bass_kernel_spmd

F32 = mybir.dt.float32
NC_ = 8
H = 1024
V = 50257
S = 4096
P = 128
T = 393            # padded vocab tiles: V_pad = 128*393 = 50304
V_PAD = P * T
GROUPS = [list(range(NC_))]
W_CHUNKS = [32] * 12 + [9]   # sum = 393
NEG_BIG = -1.0e30

AF = mybir.ActivationFunctionType


def build_nc(finalize=True):
    nc = bacc.Bacc("TRN2", target_bir_lowering=False, debug=False, num_devices=NC_)

    def din(name, shape):
        return nc.dram_tensor(name, shape, F32, kind="ExternalInput")

    wout_h = din("wout", [2, P, T, P])
    t0_h = din("t0", [P, 48, P])
    t0h_h = din("t0h", [P, 24, P])
    a1_h = din("a1", [P, 24, P])
    a2_h = din("a2", [P, 24, P])
    a3_h = din("a3", [P, 8, P])
    a4_h = din("a4", [P, 32, P])
    a5_h = din("a5", [P, 32, P])
    rnn_h = din("rnn_in_t", [P, 16])
    hp0t_h = din("hp0_t", [P, 8])
    hp1t_h = din("hp1_t", [P, 8])
    hp0s_h = din("hp0_s", [P, 1])
    hp1s_h = din("hp1_s", [P, 1])
    hsel_h = din("hsel", [P, 8])
    b0i_h = din("b0i", [P, 3])
    b0h_h = din("b0h", [P, 3])
    b1i_h = din("b1i", [P, 24])
    b1h_h = din("b1h", [P, 24])
    barr_h = din("barr", [P, T])

    out_logp_h = nc.dram_tensor("out_logp", [P, T], F32, kind="ExternalOutput")
    out_ctx_h = nc.dram_tensor("out_ctx", [P, 1], F32, kind="ExternalOutput")
    out_h0_h = nc.dram_tensor("out_h0", [P, 8], F32, kind="ExternalOutput")
    out_h1_h = nc.dram_tensor("out_h1", [P, 8], F32, kind="ExternalOutput")
    out_attn_h = nc.dram_tensor("out_attn", [P, 32], F32, kind="ExternalOutput")

    with tile.TileContext(nc) as tc:
        with (
            tc.tile_pool(name="sb", bufs=1) as sb,
            tc.tile_pool(name="wp", bufs=4) as wp,
            tc.tile_pool(name="ps", bufs=1, space="PSUM") as ps,
            tc.tile_pool(name="psg", bufs=2, space="PSUM") as psg,
            tc.tile_pool(name="dram", bufs=1, space="DRAM") as dram,
        ):
            nc.gpsimd.load_library(library_config.mlp)

            # ---- warmup collective: absorbs the cold-start latency of the
            # collective engine while the big weight DMAs stream in.
            wz = sb.tile([1, 1], F32, tag="wz")
            nc.vector.memset(wz[:], 0.0)
            warm_in = dram.tile([1, 1], F32, tag="warm_in")
            warm_out = dram.tile([1, 1], F32, tag="warm_out")
            nc.sync.dma_start(warm_in[:], wz[:])
            nc.gpsimd.collective_compute(
                "AllReduce", mybir.AluOpType.add, replica_groups=GROUPS,
                ins=[warm_in.opt()], outs=[warm_out.opt()],
            )

            # ---- small resident inputs
            def load(h, shape, tag):
                t_ = sb.tile(shape, F32, tag=tag, name=tag)
                nc.sync.dma_start(t_[:], h[:])
                return t_

            rnn_sb = load(rnn_h, [P, 16], "rnn_sb")
            hp0t_sb = load(hp0t_h, [P, 8], "hp0t_sb")
            hp1t_sb = load(hp1t_h, [P, 8], "hp1t_sb")
            hp0s_sb = load(hp0s_h, [P, 1], "hp0s_sb")
            hp1s_sb = load(hp1s_h, [P, 1], "hp1s_sb")
            hsel_sb = load(hsel_h, [P, 8], "hsel_sb")
            b0i_sb = load(b0i_h, [P, 3], "b0i_sb")
            b0h_sb = load(b0h_h, [P, 3], "b0h_sb")
            b1i_sb = load(b1i_h, [P, 24], "b1i_sb")
            b1h_sb = load(b1h_h, [P, 24], "b1h_sb")
            t0_sb = load(t0_h, [P, 48, P], "t0_sb")
            t0h_sb = load(t0h_h, [P, 24, P], "t0h_sb")
            a1_sb = load(a1_h, [P, 24, P], "a1_sb")
            a2_sb = load(a2_h, [P, 24, P], "a2_sb")
            a3_sb = load(a3_h, [P, 8, P], "a3_sb")
            a4_sb = load(a4_h, [P, 32, P], "a4_sb")
            a5_sb = load(a5_h, [P, 32, P], "a5_sb")
            barr_sb = load(barr_h, [P, T], "barr_sb")

            # ---- GRU layer 0 (output-sharded: this core's 128 units)
            g0i_ps = psg.tile([P, 24], F32, tag="g0", name="g0i_ps")
            g0h_ps = psg.tile([P, 24], F32, tag="g0", name="g0h_ps")
            for g in range(3):
                for kc in range(16):
                    j = kc * 3 + g
                    nc.tensor.matmul(
                        g0i_ps[:, g:g + 1], t0_sb[:, j, :], rnn_sb[:, kc:kc + 1],
                        start=(kc == 0), stop=(kc == 15))
            for g in range(3):
                for kc in range(8):
                    j = kc * 3 + g
                    nc.tensor.matmul(
                        g0h_ps[:, g:g + 1], t0h_sb[:, j, :], hp0t_sb[:, kc:kc + 1],
                        start=(kc == 0), stop=(kc == 7))

            g0i_b = sb.tile([P, 3], F32, tag="g0i_b")
            g0h_b = sb.tile([P, 3], F32, tag="g0h_b")
            nc.vector.tensor_add(g0i_b[:], g0i_ps[:, 0:3], b0i_sb[:])
            nc.vector.tensor_add(g0h_b[:], g0h_ps[:, 0:3], b0h_sb[:])
            r0p = sb.tile([P, 1], F32, tag="r0p")
            nc.vector.tensor_add(r0p[:], g0i_b[:, 0:1], g0h_b[:, 0:1])
            r0 = sb.tile([P, 1], F32, tag="r0")
            nc.scalar.activation(r0[:], r0p[:], AF.Sigmoid)
            z0p = sb.tile([P, 1], F32, tag="z0p")
            nc.vector.tensor_add(z0p[:], g0i_b[:, 1:2], g0h_b[:, 1:2])
            z0 = sb.tile([P, 1], F32, tag="z0")
            nc.scalar.activation(z0[:], z0p[:], AF.Sigmoid)
            n0a = sb.tile([P, 1], F32, tag="n0a")
            nc.vector.tensor_mul(n0a[:], r0[:], g0h_b[:, 2:3])
            n0b = sb.tile([P, 1], F32, tag="n0b")
            nc.vector.tensor_add(n0b[:], n0a[:], g0i_b[:, 2:3])
            n0 = sb.tile([P, 1], F32, tag="n0")
            nc.scalar.activation(n0[:], n0b[:], AF.Tanh)
            d0 = sb.tile([P, 1], F32, tag="d0")
            nc.vector.tensor_sub(d0[:], hp0s_sb[:], n0[:])
            e0 = sb.tile([P, 1], F32, tag="e0")
            nc.vector.tensor_mul(e0[:], z0[:], d0[:])
            h0s = sb.tile([P, 1], F32, tag="h0s")
            nc.vector.tensor_add(h0s[:], n0[:], e0[:])

            # ---- GRU layer 1 partial matmuls (contraction-sharded)
            g1i_ps = psg.tile([P, 24], F32, tag="g0", name="g1i_ps")
            g1h_ps = psg.tile([P, 24], F32, tag="g0", name="g1h_ps")
            for c in range(24):
                nc.tensor.matmul(g1i_ps[:, c:c + 1], a1_sb[:, c, :], h0s[:],
                                 start=True, stop=True)
            for c in range(24):
                nc.tensor.matmul(g1h_ps[:, c:c + 1], a2_sb[:, c, :], hp1s_sb[:],
                                 start=True, stop=True)

            # fused AllReduce payload: [gi1 | gh1 | h0 one-hot-padded]
            ccpay = sb.tile([P, 56], F32, tag="ccpay")
            nc.vector.tensor_copy(ccpay[:, 0:24], g1i_ps[:])
            nc.vector.tensor_copy(ccpay[:, 24:48], g1h_ps[:])
            nc.vector.tensor_scalar_mul(ccpay[:, 48:56], hsel_sb[:], h0s[:])
            cc1_in = dram.tile([P, 56], F32, tag="cc1_in")
            cc1_out = dram.tile([P, 56], F32, tag="cc1_out")
            nc.sync.dma_start(cc1_in[:], ccpay[:])
            nc.gpsimd.collective_compute(
                "AllReduce", mybir.AluOpType.add, replica_groups=GROUPS,
                ins=[cc1_in.opt()], outs=[cc1_out.opt()],
            )
            cc1_sb = sb.tile([P, 56], F32, tag="cc1_sb")
            nc.sync.dma_start(cc1_sb[:], cc1_out[:])
            nc.sync.dma_start(out_h0_h[:], cc1_sb[:, 48:56])

            # ---- GRU layer 1 gates (replicated on every core)
            g1i_b = sb.tile([P, 24], F32, tag="g1i_b")
            g1h_b = sb.tile([P, 24], F32, tag="g1h_b")
            nc.vector.tensor_add(g1i_b[:], cc1_sb[:, 0:24], b1i_sb[:])
            nc.vector.tensor_add(g1h_b[:], cc1_sb[:, 24:48], b1h_sb[:])
            r1p = sb.tile([P, 8], F32, tag="r1p")
            nc.vector.tensor_add(r1p[:], g1i_b[:, 0:8], g1h_b[:, 0:8])
            r1 = sb.tile([P, 8], F32, tag="r1")
            nc.scalar.activation(r1[:], r1p[:], AF.Sigmoid)
            z1p = sb.tile([P, 8], F32, tag="z1p")
            nc.vector.tensor_add(z1p[:], g1i_b[:, 8:16], g1h_b[:, 8:16])
            z1 = sb.tile([P, 8], F32, tag="z1")
            nc.scalar.activation(z1[:], z1p[:], AF.Sigmoid)
            n1a = sb.tile([P, 8], F32, tag="n1a")
            nc.vector.tensor_mul(n1a[:], r1[:], g1h_b[:, 16:24])
            n1b = sb.tile([P, 8], F32, tag="n1b")
            nc.vector.tensor_add(n1b[:], n1a[:], g1i_b[:, 16:24])
            n1 = sb.tile([P, 8], F32, tag="n1")
            nc.scalar.activation(n1[:], n1b[:], AF.Tanh)
            d1 = sb.tile([P, 8], F32, tag="d1")
            nc.vector.tensor_sub(d1[:], hp1t_sb[:], n1[:])
            e1t = sb.tile([P, 8], F32, tag="e1t")
            nc.vector.tensor_mul(e1t[:], z1[:], d1[:])
            h1_sb = sb.tile([P, 8], F32, tag="h1_sb")
            nc.vector.tensor_add(h1_sb[:], n1[:], e1t[:])
            nc.sync.dma_start(out_h1_h[:], h1_sb[:])

            # rhs0[k] = h1[i*128 + k]  (data-driven core-local column select)
            rsel = sb.tile([P, 8], F32, tag="rsel")
            nc.vector.tensor_mul(rsel[:], h1_sb[:], hsel_sb[:])
            rhs0 = sb.tile([P, 1], F32, tag="rhs0")
            nc.vector.reduce_sum(rhs0[:], rsel[:], axis=mybir.AxisListType.X)

            # ---- attention: u shard, partial energies
            u_ps = ps.tile([P, 1], F32, tag="aps", name="u_ps")
            for hc in range(8):
                nc.tensor.matmul(u_ps[:], a3_sb[:, hc, :], h1_sb[:, hc:hc + 1],
                                 start=(hc == 0), stop=(hc == 7))
            u_sb = sb.tile([P, 1], F32, tag="u_sb")
            nc.vector.tensor_copy(u_sb[:], u_ps[:])
            e_ps = ps.tile([P, 32], F32, tag="aps", name="e_ps")
            for t in range(32):
                nc.tensor.matmul(e_ps[:, t:t + 1], a4_sb[:, t, :], u_sb[:],
                                 start=True, stop=True)
            e_in = sb.tile([P, 32], F32, tag="e_in")
            nc.vector.tensor_copy(e_in[:], e_ps[:])
            cc2_in = dram.tile([P, 32], F32, tag="cc2_in")
            cc2_out = dram.tile([P, 32], F32, tag="cc2_out")
            nc.sync.dma_start(cc2_in[:], e_in[:])
            nc.gpsimd.collective_compute(
                "AllReduce", mybir.AluOpType.add, replica_groups=GROUPS,
                ins=[cc2_in.opt()], outs=[cc2_out.opt()],
            )
            e_full = sb.tile([P, 32], F32, tag="e_full")
            nc.sync.dma_start(e_full[:], cc2_out[:])

            # ---- logits r-part can overlap the AR2 wait on PE
            ps_r = ps.tile([P, T], F32, tag="ps_r")
            ps_c = ps.tile([P, T], F32, tag="ps_c")
            t0c = 0
            for tb in W_CHUNKS:
                wt = wp.tile([P, 32, P], F32, tag="wch", name=f"wtr{t0c}")
                nc.sync.dma_start(wt[:, :tb, :], wout_h[0, :, t0c:t0c + tb, :])
                for tt in range(tb):
                    t = t0c + tt
                    nc.tensor.matmul(ps_r[:, t:t + 1], wt[:, tt, :], rhs0[:],
                                     start=True, stop=True)
                t0c += tb

            # ---- softmax over energies (replicated)
            m1 = sb.tile([P, 1], F32, tag="m1")
            nc.vector.reduce_max(m1[:], e_full[:], axis=mybir.AxisListType.X)
            gm = sb.tile([P, 1], F32, tag="gm")
            nc.gpsimd.partition_all_reduce(gm[:], m1[:], 128, bass_isa.ReduceOp.max)
            ngm = sb.tile([P, 1], F32, tag="ngm")
            nc.vector.tensor_scalar_mul(ngm[:], gm[:], -1.0)
            pexp = sb.tile([P, 32], F32, tag="pexp")
            se = sb.tile([P, 1], F32, tag="se")
            nc.scalar.activation(pexp[:], e_full[:], AF.Exp, bias=ngm[:],
                                 accum_out=se[:])
            gs = sb.tile([P, 1], F32, tag="gs")
            nc.gpsimd.partition_all_reduce(gs[:], se[:], 128, bass_isa.ReduceOp.add)
            ri = sb.tile([P, 1], F32, tag="ri")
            nc.vector.reciprocal(ri[:], gs[:])
            p_sb = sb.tile([P, 32], F32, tag="p_sb")
            nc.vector.tensor_scalar_mul(p_sb[:], pexp[:], ri[:])
            nc.sync.dma_start(out_attn_h[:], p_sb[:])

            # ---- context shard (no collective needed)
            ctx_ps = ps.tile([P, 1], F32, tag="aps", name="ctx_ps")
            for t in range(32):
                nc.tensor.matmul(ctx_ps[:], a5_sb[:, t, :], p_sb[:, t:t + 1],
                                 start=(t == 0), stop=(t == 31))
            ctx_sb = sb.tile([P, 1], F32, tag="ctx_sb")
            nc.vector.tensor_copy(ctx_sb[:], ctx_ps[:])
            nc.sync.dma_start(out_ctx_h[:], ctx_sb[:])

            # ---- logits ctx-part
            t0c = 0
            for tb in W_CHUNKS:
                wt = wp.tile([P, 32, P], F32, tag="wch", name=f"wtc{t0c}")
                nc.sync.dma_start(wt[:, :tb, :], wout_h[1, :, t0c:t0c + tb, :])
                for tt in range(tb):
                    t = t0c + tt
                    nc.tensor.matmul(ps_c[:, t:t + 1], wt[:, tt, :], ctx_sb[:],
                                     start=True, stop=True)
                t0c += tb

            evc = sb.tile([P, T], F32, tag="evc")
            nc.vector.tensor_copy(evc[:], ps_c[:])
            ev = sb.tile([P, T], F32, tag="ev")
            nc.vector.tensor_add(ev[:], ps_r[:], evc[:])
            cc3_in = dram.tile([P, T], F32, tag="cc3_in")
            cc3_out = dram.tile([P, T], F32, tag="cc3_out")
            nc.sync.dma_start(cc3_in[:], ev[:])
            nc.gpsimd.collective_compute(
                "AllReduce", mybir.AluOpType.add, replica_groups=GROUPS,
                ins=[cc3_in.opt()], outs=[cc3_out.opt()],
            )
            cc3_sb = sb.tile([P, T], F32, tag="cc3_sb")
            nc.sync.dma_start(cc3_sb[:], cc3_out[:])

            # ---- log-softmax (replicated)
            lf = sb.tile([P, T], F32, tag="lf")
            nc.vector.tensor_add(lf[:], cc3_sb[:], barr_sb[:])
            m2 = sb.tile([P, 1], F32, tag="m2")
            nc.vector.reduce_max(m2[:], lf[:], axis=mybir.AxisListType.X)
            gm2 = sb.tile([P, 1], F32, tag="gm2")
            nc.gpsimd.partition_all_reduce(gm2[:], m2[:], 128, bass_isa.ReduceOp.max)
            ngm2 = sb.tile([P, 1], F32, tag="ngm2")
            nc.vector.tensor_scalar_mul(ngm2[:], gm2[:], -1.0)
            pex2 = sb.tile([P, T], F32, tag="pex2")
            se2 = sb.tile([P, 1], F32, tag="se2")
            nc.scalar.activation(pex2[:], lf[:], AF.Exp, bias=ngm2[:],
                                 accum_out=se2[:])
            gs2 = sb.tile([P, 1], F32, tag="gs2")
            nc.gpsimd.partition_all_reduce(gs2[:], se2[:], 128, bass_isa.ReduceOp.add)
            l2 = sb.tile([P, 1], F32, tag="l2")
            nc.scalar.activation(l2[:], gs2[:], AF.Ln)
            logz = sb.tile([P, 1], F32, tag="logz")
            nc.vector.tensor_add(logz[:], gm2[:], l2[:])
            outl = sb.tile([P, T], F32, tag="outl")
            nc.vector.tensor_scalar_sub(outl[:], lf[:], logz[:])
            nc.sync.dma_start(out_logp_h[:], outl[:])

    if finalize:
        nc.finalize()
    return nc


def prep_in_maps(inputs):
    """Shard the full (unsharded) problem inputs into 8 per-core input maps."""
    f = np.float32

    def arr(name):
        return np.asarray(inputs[name], f)

    emb_row = np.asarray(inputs["emb"][int(np.asarray(inputs["word_input"]).ravel()[0])], f)
    rnn_in = np.concatenate([emb_row.ravel(), arr("last_context").ravel()])
    rnn_in_t = np.ascontiguousarray(rnn_in.reshape(16, P).T)
    lh = arr("last_hidden")
    hp0, hp1 = lh[0, 0], lh[1, 0]
    hp0_t = np.ascontiguousarray(hp0.reshape(8, P).T)
    hp1_t = np.ascontiguousarray(hp1.reshape(8, P).T)

    t0_all = arr("W_ih0").reshape(3, 8, P, 16, P).transpose(1, 4, 3, 0, 2)
    t0h_all = arr("W_hh0").reshape(3, 8, P, 8, P).transpose(1, 4, 3, 0, 2)
    a1_all = arr("W_ih1").reshape(24, P, 8, P).transpose(2, 3, 0, 1)
    a2_all = arr("W_hh1").reshape(24, P, 8, P).transpose(2, 3, 0, 1)
    a3_all = arr("W_attn").reshape(8, P, 8, P).transpose(2, 1, 0, 3)
    enc = arr("encoder_outputs")[:, 0, :]
    e4 = enc.reshape(32, P, 8, P)
    a4_all = e4.transpose(2, 3, 0, 1)
    a5_all = e4.transpose(2, 1, 0, 3)

    wp_ = np.zeros((V_PAD, 2 * H), f)
    wp_[:V] = arr("W_out")
    wall = np.ascontiguousarray(wp_.reshape(P, T, 16, P).transpose(2, 3, 1, 0))

    bp = np.full(V_PAD, NEG_BIG, f)
    bp[:V] = arr("b_out")
    barr = bp.reshape(P, T)

    b0i_all = arr("b_ih0").reshape(3, 8, P).transpose(1, 2, 0)
    b0h_all = arr("b_hh0").reshape(3, 8, P).transpose(1, 2, 0)
    b1i = np.ascontiguousarray(arr("b_ih1").reshape(24, P).T)
    b1h = np.ascontiguousarray(arr("b_hh1").reshape(24, P).T)

    in_maps = []
    for i in range(NC_):
        hsel = np.zeros((P, 8), f)
        hsel[:, i] = 1.0
        in_maps.append({
            "wout": np.ascontiguousarray(wall[[i, 8 + i]]),
            "t0": np.ascontiguousarray(t0_all[i].reshape(P, 48, P)),
            "t0h": np.ascontiguousarray(t0h_all[i].reshape(P, 24, P)),
            "a1": np.ascontiguousarray(a1_all[i]),
            "a2": np.ascontiguousarray(a2_all[i]),
            "a3": np.ascontiguousarray(a3_all[i]),
            "a4": np.ascontiguousarray(a4_all[i]),
            "a5": np.ascontiguousarray(a5_all[i]),
            "rnn_in_t": rnn_in_t,
            "hp0_t": hp0_t,
            "hp1_t": hp1_t,
            "hp0_s": np.ascontiguousarray(hp0[i * P:(i + 1) * P].reshape(P, 1)),
            "hp1_s": np.ascontiguousarray(hp1[i * P:(i + 1) * P].reshape(P, 1)),
            "hsel": hsel,
            "b0i": np.ascontiguousarray(b0i_all[i]),
            "b0h": np.ascontiguousarray(b0h_all[i]),
            "b1i": b1i,
            "b1h": b1h,
            "barr": barr,
        })
    return in_maps


def unpack_outputs(results):
    out_logp = np.asarray(results[0]["out_logp"], np.float32).ravel()[:V].reshape(1, V)
    ctx = np.concatenate(
        [np.asarray(results[i]["out_ctx"], np.float32).ravel() for i in range(NC_)]
    ).reshape(1, H)
    h0 = np.asarray(results[0]["out_h0"], np.float32).T.ravel()
    h1 = np.asarray(results[0]["out_h1"], np.float32).T.ravel()
    hidden = np.stack([h0, h1]).reshape(2, 1, H)
    attn = np.asarray(results[0]["out_attn"], np.float32).T.ravel().reshape(1, 1, S)
    return out_logp, ctx, hidden, attn


_NC_CACHE = None


def run_on_hw(inputs, trace=False):
    global _NC_CACHE
    if _NC_CACHE is None:
        _NC_CACHE = build_nc()
    in_maps = prep_in_maps(inputs)
    res = run_bass_kernel_spmd(_NC_CACHE, in_maps, list(range(NC_)), trace=trace)
    return unpack_outputs(res.results), res


def kernel(**inputs):
    outs, _ = run_on_hw(inputs, trace=False)
    return outs


# revision 16
# speedup vs baseline: 1.0821x; 1.0821x over previous
"""AttnDecoderRNN single-step decoder on 8 Trainium2 NeuronCores.

Tensor-parallel sharding (hardcoded, 8 cores):
  - GRU layer 0: hidden units sharded (128 per core), full contraction.
  - GRU layer 1: contraction sharded; partial (gi1|gh1|h0) fused AllReduce.
  - Attention: W_attn/encoder columns sharded; partial-energy AllReduce;
    softmax replicated; context column-sharded (no collective).
  - Output projection: contraction (2H) sharded 256/core; partial-logits
    AllReduce; log-softmax replicated locally.
Embedding row gather happens on host (only one row of emb is ever read).
"""

import numpy as np

import concourse.bacc as bacc
import concourse.bass as bass
import concourse.mybir as mybir
import concourse.tile as tile
from concourse import bass_isa
from concourse import library_config
from concourse.bass_utils import run_bass_kernel_spmd

F32 = mybir.dt.float32
NC_ = 8
H = 1024
V = 50257
S = 4096
P = 128
T = 393            # padded vocab tiles: V_pad = 128*393 = 50304
V_PAD = P * T
GROUPS = [list(range(NC_))]
W_CHUNKS = [32] * 12 + [9]   # sum = 393
NEG_BIG = -1.0e30
W_DVE_CHUNKS = [8] * 49 + [1]   # sum = 393

AF = mybir.ActivationFunctionType


def build_nc(finalize=True):
    nc = bacc.Bacc("TRN2", target_bir_lowering=False, debug=False, num_devices=NC_)

    def din(name, shape):
        return nc.dram_tensor(name, shape, F32, kind="ExternalInput")

    wdve_h = din("wdve", [P, T, 256])
    ident_h = din("ident", [P, P])
    t0_h = din("t0", [P, 48, P])
    t0h_h = din("t0h", [P, 24, P])
    a1_h = din("a1", [P, 24, P])
    a2_h = din("a2", [P, 24, P])
    a3_h = din("a3", [P, 8, P])
    a4_h = din("a4", [P, 32, P])
    a5_h = din("a5", [P, 32, P])
    rnn_h = din("rnn_in_t", [P, 16])
    hp0t_h = din("hp0_t", [P, 8])
    hp1t_h = din("hp1_t", [P, 8])
    hp0s_h = din("hp0_s", [P, 1])
    hp1s_h = din("hp1_s", [P, 1])
    hsel_h = din("hsel", [P, 8])
    b0i_h = din("b0i", [P, 3])
    b0h_h = din("b0h", [P, 3])
    b1i_h = din("b1i", [P, 24])
    b1h_h = din("b1h", [P, 24])
    barr_h = din("barr", [P, T])

    out_logp_h = nc.dram_tensor("out_logp", [P, T], F32, kind="ExternalOutput")
    out_ctx_h = nc.dram_tensor("out_ctx", [P, 1], F32, kind="ExternalOutput")
    out_h0_h = nc.dram_tensor("out_h0", [P, 8], F32, kind="ExternalOutput")
    out_h1_h = nc.dram_tensor("out_h1", [P, 8], F32, kind="ExternalOutput")
    out_attn_h = nc.dram_tensor("out_attn", [P, 32], F32, kind="ExternalOutput")

    with tile.TileContext(nc) as tc:
        with (
            tc.tile_pool(name="sb", bufs=1) as sb,
            tc.tile_pool(name="wp", bufs=6) as wp,
            tc.tile_pool(name="pp", bufs=2) as pp,
            tc.tile_pool(name="ps", bufs=1, space="PSUM") as ps,
            tc.tile_pool(name="psg", bufs=2, space="PSUM") as psg,
            tc.tile_pool(name="dram", bufs=1, space="DRAM") as dram,
        ):
            nc.gpsimd.load_library(library_config.mlp)

            # ---- warmup collective: absorbs the cold-start latency of the
            # collective engine while the big weight DMAs stream in.
            wz = sb.tile([1, 1], F32, tag="wz")
            nc.vector.memset(wz[:], 0.0)
            warm_in = dram.tile([1, 1], F32, tag="warm_in")
            warm_out = dram.tile([1, 1], F32, tag="warm_out")
            nc.sync.dma_start(warm_in[:], wz[:])
            nc.gpsimd.collective_compute(
                "AllReduce", mybir.AluOpType.add, replica_groups=GROUPS,
                ins=[warm_in.opt()], outs=[warm_out.opt()],
            )

            # ---- small resident inputs
            def load(h, shape, tag):
                t_ = sb.tile(shape, F32, tag=tag, name=tag)
                nc.sync.dma_start(t_[:], h[:])
                return t_

            rnn_sb = load(rnn_h, [P, 16], "rnn_sb")
            hp0t_sb = load(hp0t_h, [P, 8], "hp0t_sb")
            hp1t_sb = load(hp1t_h, [P, 8], "hp1t_sb")
            hp0s_sb = load(hp0s_h, [P, 1], "hp0s_sb")
            hp1s_sb = load(hp1s_h, [P, 1], "hp1s_sb")
            hsel_sb = load(hsel_h, [P, 8], "hsel_sb")
            b0i_sb = load(b0i_h, [P, 3], "b0i_sb")
            b0h_sb = load(b0h_h, [P, 3], "b0h_sb")
            b1i_sb = load(b1i_h, [P, 24], "b1i_sb")
            b1h_sb = load(b1h_h, [P, 24], "b1h_sb")
            t0_sb = load(t0_h, [P, 48, P], "t0_sb")
            t0h_sb = load(t0h_h, [P, 24, P], "t0h_sb")
            a1_sb = load(a1_h, [P, 24, P], "a1_sb")
            a2_sb = load(a2_h, [P, 24, P], "a2_sb")
            a3_sb = load(a3_h, [P, 8, P], "a3_sb")
            a4_sb = load(a4_h, [P, 32, P], "a4_sb")
            a5_sb = load(a5_h, [P, 32, P], "a5_sb")
            barr_sb = load(barr_h, [P, T], "barr_sb")
            ident_sb = load(ident_h, [P, P], "ident_sb")

            # ---- GRU layer 0 (output-sharded: this core's 128 units)
            g0i_ps = psg.tile([P, 24], F32, tag="g0", name="g0i_ps")
            g0h_ps = psg.tile([P, 24], F32, tag="g0", name="g0h_ps")
            for g in range(3):
                for kc in range(16):
                    j = kc * 3 + g
                    nc.tensor.matmul(
                        g0i_ps[:, g:g + 1], t0_sb[:, j, :], rnn_sb[:, kc:kc + 1],
                        start=(kc == 0), stop=(kc == 15))
            for g in range(3):
                for kc in range(8):
                    j = kc * 3 + g
                    nc.tensor.matmul(
                        g0h_ps[:, g:g + 1], t0h_sb[:, j, :], hp0t_sb[:, kc:kc + 1],
                        start=(kc == 0), stop=(kc == 7))

            g0i_b = sb.tile([P, 3], F32, tag="g0i_b")
            g0h_b = sb.tile([P, 3], F32, tag="g0h_b")
            nc.vector.tensor_add(g0i_b[:], g0i_ps[:, 0:3], b0i_sb[:])
            nc.vector.tensor_add(g0h_b[:], g0h_ps[:, 0:3], b0h_sb[:])
            r0p = sb.tile([P, 1], F32, tag="r0p")
            nc.vector.tensor_add(r0p[:], g0i_b[:, 0:1], g0h_b[:, 0:1])
            r0 = sb.tile([P, 1], F32, tag="r0")
            nc.scalar.activation(r0[:], r0p[:], AF.Sigmoid)
            z0p = sb.tile([P, 1], F32, tag="z0p")
            nc.vector.tensor_add(z0p[:], g0i_b[:, 1:2], g0h_b[:, 1:2])
            z0 = sb.tile([P, 1], F32, tag="z0")
            nc.scalar.activation(z0[:], z0p[:], AF.Sigmoid)
            n0a = sb.tile([P, 1], F32, tag="n0a")
            nc.vector.tensor_mul(n0a[:], r0[:], g0h_b[:, 2:3])
            n0b = sb.tile([P, 1], F32, tag="n0b")
            nc.vector.tensor_add(n0b[:], n0a[:], g0i_b[:, 2:3])
            n0 = sb.tile([P, 1], F32, tag="n0")
            nc.scalar.activation(n0[:], n0b[:], AF.Tanh)
            d0 = sb.tile([P, 1], F32, tag="d0")
            nc.vector.tensor_sub(d0[:], hp0s_sb[:], n0[:])
            e0 = sb.tile([P, 1], F32, tag="e0")
            nc.vector.tensor_mul(e0[:], z0[:], d0[:])
            h0s = sb.tile([P, 1], F32, tag="h0s")
            nc.vector.tensor_add(h0s[:], n0[:], e0[:])

            # ---- GRU layer 1 partial matmuls (contraction-sharded)
            g1i_ps = psg.tile([P, 24], F32, tag="g0", name="g1i_ps")
            g1h_ps = psg.tile([P, 24], F32, tag="g0", name="g1h_ps")
            for c in range(24):
                nc.tensor.matmul(g1i_ps[:, c:c + 1], a1_sb[:, c, :], h0s[:],
                                 start=True, stop=True)
            for c in range(24):
                nc.tensor.matmul(g1h_ps[:, c:c + 1], a2_sb[:, c, :], hp1s_sb[:],
                                 start=True, stop=True)

            # fused AllReduce payload: [gi1 | gh1 | h0 one-hot-padded]
            ccpay = sb.tile([P, 56], F32, tag="ccpay")
            nc.vector.tensor_copy(ccpay[:, 0:24], g1i_ps[:])
            nc.vector.tensor_copy(ccpay[:, 24:48], g1h_ps[:])
            nc.vector.tensor_scalar_mul(ccpay[:, 48:56], hsel_sb[:], h0s[:])
            cc1_in = dram.tile([P, 56], F32, tag="cc1_in")
            cc1_out = dram.tile([P, 56], F32, tag="cc1_out")
            nc.sync.dma_start(cc1_in[:], ccpay[:])
            nc.gpsimd.collective_compute(
                "AllReduce", mybir.AluOpType.add, replica_groups=GROUPS,
                ins=[cc1_in.opt()], outs=[cc1_out.opt()],
            )
            cc1_sb = sb.tile([P, 56], F32, tag="cc1_sb")
            nc.sync.dma_start(cc1_sb[:], cc1_out[:])
            nc.sync.dma_start(out_h0_h[:], cc1_sb[:, 48:56])

            # ---- GRU layer 1 gates (replicated on every core)
            g1i_b = sb.tile([P, 24], F32, tag="g1i_b")
            g1h_b = sb.tile([P, 24], F32, tag="g1h_b")
            nc.vector.tensor_add(g1i_b[:], cc1_sb[:, 0:24], b1i_sb[:])
            nc.vector.tensor_add(g1h_b[:], cc1_sb[:, 24:48], b1h_sb[:])
            r1p = sb.tile([P, 8], F32, tag="r1p")
            nc.vector.tensor_add(r1p[:], g1i_b[:, 0:8], g1h_b[:, 0:8])
            r1 = sb.tile([P, 8], F32, tag="r1")
            nc.scalar.activation(r1[:], r1p[:], AF.Sigmoid)
            z1p = sb.tile([P, 8], F32, tag="z1p")
            nc.vector.tensor_add(z1p[:], g1i_b[:, 8:16], g1h_b[:, 8:16])
            z1 = sb.tile([P, 8], F32, tag="z1")
            nc.scalar.activation(z1[:], z1p[:], AF.Sigmoid)
            n1a = sb.tile([P, 8], F32, tag="n1a")
            nc.vector.tensor_mul(n1a[:], r1[:], g1h_b[:, 16:24])
            n1b = sb.tile([P, 8], F32, tag="n1b")
            nc.vector.tensor_add(n1b[:], n1a[:], g1i_b[:, 16:24])
            n1 = sb.tile([P, 8], F32, tag="n1")
            nc.scalar.activation(n1[:], n1b[:], AF.Tanh)
            d1 = sb.tile([P, 8], F32, tag="d1")
            nc.vector.tensor_sub(d1[:], hp1t_sb[:], n1[:])
            e1t = sb.tile([P, 8], F32, tag="e1t")
            nc.vector.tensor_mul(e1t[:], z1[:], d1[:])
            h1_sb = sb.tile([P, 8], F32, tag="h1_sb")
            nc.vector.tensor_add(h1_sb[:], n1[:], e1t[:])
            nc.sync.dma_start(out_h1_h[:], h1_sb[:])

            # rhs0[k] = h1[i*128 + k]  (data-driven core-local column select)
            rsel = sb.tile([P, 8], F32, tag="rsel")
            nc.vector.tensor_mul(rsel[:], h1_sb[:], hsel_sb[:])
            rhs0 = sb.tile([P, 1], F32, tag="rhs0")
            nc.vector.reduce_sum(rhs0[:], rsel[:], axis=mybir.AxisListType.X)

            # ---- attention: u shard, partial energies
            u_ps = ps.tile([P, 1], F32, tag="aps", name="u_ps")
            for hc in range(8):
                nc.tensor.matmul(u_ps[:], a3_sb[:, hc, :], h1_sb[:, hc:hc + 1],
                                 start=(hc == 0), stop=(hc == 7))
            u_sb = sb.tile([P, 1], F32, tag="u_sb")
            nc.vector.tensor_copy(u_sb[:], u_ps[:])
            e_ps = ps.tile([P, 32], F32, tag="aps", name="e_ps")
            for t in range(32):
                nc.tensor.matmul(e_ps[:, t:t + 1], a4_sb[:, t, :], u_sb[:],
                                 start=True, stop=True)
            e_in = sb.tile([P, 32], F32, tag="e_in")
            nc.vector.tensor_copy(e_in[:], e_ps[:])
            cc2_in = dram.tile([P, 32], F32, tag="cc2_in")
            cc2_out = dram.tile([P, 32], F32, tag="cc2_out")
            nc.sync.dma_start(cc2_in[:], e_in[:])
            nc.gpsimd.collective_compute(
                "AllReduce", mybir.AluOpType.add, replica_groups=GROUPS,
                ins=[cc2_in.opt()], outs=[cc2_out.opt()],
            )
            e_full = sb.tile([P, 32], F32, tag="e_full")
            nc.sync.dma_start(e_full[:], cc2_out[:])

            # ---- softmax over energies (replicated)
            m1 = sb.tile([P, 1], F32, tag="m1")
            nc.vector.reduce_max(m1[:], e_full[:], axis=mybir.AxisListType.X)
            gm = sb.tile([P, 1], F32, tag="gm")
            nc.gpsimd.partition_all_reduce(gm[:], m1[:], 128, bass_isa.ReduceOp.max)
            ngm = sb.tile([P, 1], F32, tag="ngm")
            nc.vector.tensor_scalar_mul(ngm[:], gm[:], -1.0)
            pexp = sb.tile([P, 32], F32, tag="pexp")
            se = sb.tile([P, 1], F32, tag="se")
            nc.scalar.activation(pexp[:], e_full[:], AF.Exp, bias=ngm[:],
                                 accum_out=se[:])
            gs = sb.tile([P, 1], F32, tag="gs")
            nc.gpsimd.partition_all_reduce(gs[:], se[:], 128, bass_isa.ReduceOp.add)
            ri = sb.tile([P, 1], F32, tag="ri")
            nc.vector.reciprocal(ri[:], gs[:])
            p_sb = sb.tile([P, 32], F32, tag="p_sb")
            nc.vector.tensor_scalar_mul(p_sb[:], pexp[:], ri[:])
            nc.sync.dma_start(out_attn_h[:], p_sb[:])

            # ---- context shard (no collective needed)
            ctx_ps = ps.tile([P, 1], F32, tag="aps", name="ctx_ps")
            for t in range(32):
                nc.tensor.matmul(ctx_ps[:], a5_sb[:, t, :], p_sb[:, t:t + 1],
                                 start=(t == 0), stop=(t == 31))
            ctx_sb = sb.tile([P, 1], F32, tag="ctx_sb")
            nc.vector.tensor_copy(ctx_sb[:], ctx_ps[:])
            nc.sync.dma_start(out_ctx_h[:], ctx_sb[:])

            # ---- x2 = [r_shard | ctx_shard] moved to free-dim layout and
            # broadcast across partitions for the DVE matvec
            rt_ps = psg.tile([1, P], F32, tag="tp", name="rt_ps")
            nc.tensor.transpose(rt_ps[:], rhs0[:], ident_sb[:])
            ct_ps = psg.tile([1, P], F32, tag="tp", name="ct_ps")
            nc.tensor.transpose(ct_ps[:], ctx_sb[:], ident_sb[:])
            x2row = sb.tile([1, 256], F32, tag="x2row")
            nc.vector.tensor_copy(x2row[:, 0:P], rt_ps[:])
            nc.vector.tensor_copy(x2row[:, P:256], ct_ps[:])
            x2bc = sb.tile([P, 1, 256], F32, tag="x2bc")
            nc.gpsimd.partition_broadcast(x2bc[:, 0, :], x2row[:], 128)

            # ---- logits partials: DVE elementwise product per chunk, then
            # per-tile free-dim reduction on the ACT engine (accum_out)
            y_sb = sb.tile([P, T], F32, tag="y_sb")
            ascr = sb.tile([P, 256], F32, tag="ascr")
            t0c = 0
            for tb in W_DVE_CHUNKS:
                wt = wp.tile([P, 8, 256], F32, tag="wch", name=f"wt{t0c}")
                nc.sync.dma_start(wt[:, :tb, :], wdve_h[:, t0c:t0c + tb, :])
                prod = pp.tile([P, 8, 256], F32, tag="prod", name=f"prod{t0c}")
                nc.vector.tensor_mul(prod[:, :tb, :], wt[:, :tb, :],
                                     x2bc[:].broadcast_to((P, tb, 256)))
                for tt in range(tb):
                    t = t0c + tt
                    nc.scalar.activation(ascr[:], prod[:, tt, :], AF.Identity,
                                         accum_out=y_sb[:, t:t + 1])
                t0c += tb

            cc3_in = dram.tile([P, T], F32, tag="cc3_in")
            cc3_out = dram.tile([P, T], F32, tag="cc3_out")
            nc.sync.dma_start(cc3_in[:], y_sb[:])
            nc.gpsimd.collective_compute(
                "AllReduce", mybir.AluOpType.add, replica_groups=GROUPS,
                ins=[cc3_in.opt()], outs=[cc3_out.opt()],
            )
            cc3_sb = sb.tile([P, T], F32, tag="cc3_sb")
            nc.sync.dma_start(cc3_sb[:], cc3_out[:])

            # ---- log-softmax (replicated)
            lf = sb.tile([P, T], F32, tag="lf")
            nc.vector.tensor_add(lf[:], cc3_sb[:], barr_sb[:])
            m2 = sb.tile([P, 1], F32, tag="m2")
            nc.vector.reduce_max(m2[:], lf[:], axis=mybir.AxisListType.X)
            gm2 = sb.tile([P, 1], F32, tag="gm2")
            nc.gpsimd.partition_all_reduce(gm2[:], m2[:], 128, bass_isa.ReduceOp.max)
            ngm2 = sb.tile([P, 1], F32, tag="ngm2")
            nc.vector.tensor_scalar_mul(ngm2[:], gm2[:], -1.0)
            pex2 = sb.tile([P, T], F32, tag="pex2")
            se2 = sb.tile([P, 1], F32, tag="se2")
            nc.scalar.activation(pex2[:], lf[:], AF.Exp, bias=ngm2[:],
                                 accum_out=se2[:])
            gs2 = sb.tile([P, 1], F32, tag="gs2")
            nc.gpsimd.partition_all_reduce(gs2[:], se2[:], 128, bass_isa.ReduceOp.add)
            l2 = sb.tile([P, 1], F32, tag="l2")
            nc.scalar.activation(l2[:], gs2[:], AF.Ln)
            logz = sb.tile([P, 1], F32, tag="logz")
            nc.vector.tensor_add(logz[:], gm2[:], l2[:])
            outl = sb.tile([P, T], F32, tag="outl")
            nc.vector.tensor_scalar_sub(outl[:], lf[:], logz[:])
            nc.sync.dma_start(out_logp_h[:], outl[:])

    if finalize:
        nc.finalize()
    return nc


def prep_in_maps(inputs):
    """Shard the full (unsharded) problem inputs into 8 per-core input maps."""
    f = np.float32

    def arr(name):
        return np.asarray(inputs[name], f)

    emb_row = np.asarray(inputs["emb"][int(np.asarray(inputs["word_input"]).ravel()[0])], f)
    rnn_in = np.concatenate([emb_row.ravel(), arr("last_context").ravel()])
    rnn_in_t = np.ascontiguousarray(rnn_in.reshape(16, P).T)
    lh = arr("last_hidden")
    hp0, hp1 = lh[0, 0], lh[1, 0]
    hp0_t = np.ascontiguousarray(hp0.reshape(8, P).T)
    hp1_t = np.ascontiguousarray(hp1.reshape(8, P).T)

    t0_all = arr("W_ih0").reshape(3, 8, P, 16, P).transpose(1, 4, 3, 0, 2)
    t0h_all = arr("W_hh0").reshape(3, 8, P, 8, P).transpose(1, 4, 3, 0, 2)
    a1_all = arr("W_ih1").reshape(24, P, 8, P).transpose(2, 3, 0, 1)
    a2_all = arr("W_hh1").reshape(24, P, 8, P).transpose(2, 3, 0, 1)
    a3_all = arr("W_attn").reshape(8, P, 8, P).transpose(2, 1, 0, 3)
    enc = arr("encoder_outputs")[:, 0, :]
    e4 = enc.reshape(32, P, 8, P)
    a4_all = e4.transpose(2, 3, 0, 1)
    a5_all = e4.transpose(2, 1, 0, 3)

    wp_ = np.zeros((V_PAD, 2 * H), f)
    wp_[:V] = arr("W_out")
    # [t, m, kc_all, k] with v = t*128 + m
    wv = wp_.reshape(T, P, 16, P)

    bp = np.full(V_PAD, NEG_BIG, f)
    bp[:V] = arr("b_out")
    barr = np.ascontiguousarray(bp.reshape(T, P).T)
    ident = np.eye(P, dtype=f)

    b0i_all = arr("b_ih0").reshape(3, 8, P).transpose(1, 2, 0)
    b0h_all = arr("b_hh0").reshape(3, 8, P).transpose(1, 2, 0)
    b1i = np.ascontiguousarray(arr("b_ih1").reshape(24, P).T)
    b1h = np.ascontiguousarray(arr("b_hh1").reshape(24, P).T)

    in_maps = []
    for i in range(NC_):
        hsel = np.zeros((P, 8), f)
        hsel[:, i] = 1.0
        wdve = wv[:, :, (i, 8 + i), :].transpose(1, 0, 2, 3).reshape(P, T, 256)
        in_maps.append({
            "wdve": np.ascontiguousarray(wdve),
            "ident": ident,
            "t0": np.ascontiguousarray(t0_all[i].reshape(P, 48, P)),
            "t0h": np.ascontiguousarray(t0h_all[i].reshape(P, 24, P)),
            "a1": np.ascontiguousarray(a1_all[i]),
            "a2": np.ascontiguousarray(a2_all[i]),
            "a3": np.ascontiguousarray(a3_all[i]),
            "a4": np.ascontiguousarray(a4_all[i]),
            "a5": np.ascontiguousarray(a5_all[i]),
            "rnn_in_t": rnn_in_t,
            "hp0_t": hp0_t,
            "hp1_t": hp1_t,
            "hp0_s": np.ascontiguousarray(hp0[i * P:(i + 1) * P].reshape(P, 1)),
            "hp1_s": np.ascontiguousarray(hp1[i * P:(i + 1) * P].reshape(P, 1)),
            "hsel": hsel,
            "b0i": np.ascontiguousarray(b0i_all[i]),
            "b0h": np.ascontiguousarray(b0h_all[i]),
            "b1i": b1i,
            "b1h": b1h,
            "barr": barr,
        })
    return in_maps


def unpack_outputs(results):
    out_logp = np.asarray(results[0]["out_logp"], np.float32).T.ravel()[:V].reshape(1, V)
    ctx = np.concatenate(
        [np.asarray(results[i]["out_ctx"], np.float32).ravel() for i in range(NC_)]
    ).reshape(1, H)
    h0 = np.asarray(results[0]["out_h0"], np.float32).T.ravel()
    h1 = np.asarray(results[0]["out_h1"], np.float32).T.ravel()
    hidden = np.stack([h0, h1]).reshape(2, 1, H)
    attn = np.asarray(results[0]["out_attn"], np.float32).T.ravel().reshape(1, 1, S)
    return out_logp, ctx, hidden, attn


_NC_CACHE = None


def run_on_hw(inputs, trace=False):
    global _NC_CACHE
    if _NC_CACHE is None:
        _NC_CACHE = build_nc()
    in_maps = prep_in_maps(inputs)
    res = run_bass_kernel_spmd(_NC_CACHE, in_maps, list(range(NC_)), trace=trace)
    return unpack_outputs(res.results), res


def kernel(**inputs):
    outs, _ = run_on_hw(inputs, trace=False)
    return outs


# revision 21
# speedup vs baseline: 1.2396x; 1.1456x over previous
"""AttnDecoderRNN single-step decoder on 8 Trainium2 NeuronCores.

Tensor-parallel sharding (hardcoded, 8 cores):
  - GRU layer 0: hidden units sharded (128 per core), full contraction.
  - GRU layer 1: contraction sharded; partial (gi1|gh1|h0) fused AllReduce.
  - Attention: W_attn/encoder columns sharded; partial-energy AllReduce;
    softmax replicated; context column-sharded (no collective).
  - Output projection: contraction (2H) sharded 256/core; partial-logits
    AllReduce; log-softmax replicated locally.
Embedding row gather happens on host (only one row of emb is ever read).
"""

import numpy as np

import concourse.bacc as bacc
import concourse.bass as bass
import concourse.mybir as mybir
import concourse.tile as tile
from concourse import bass_isa
from concourse import library_config
from concourse.bass_utils import run_bass_kernel_spmd

F32 = mybir.dt.float32
NC_ = 8
H = 1024
V = 50257
S = 4096
P = 128
T = 393            # padded vocab tiles: V_pad = 128*393 = 50304
V_PAD = P * T
GROUPS = [list(range(NC_))]
W_CHUNKS = [32] * 12 + [9]   # sum = 393
NEG_BIG = -1.0e30
W_DVE_CHUNKS = [8] * 49 + [1]   # sum = 393

AF = mybir.ActivationFunctionType


def build_nc(finalize=True):
    nc = bacc.Bacc("TRN2", target_bir_lowering=False, debug=False, num_devices=NC_)

    def din(name, shape):
        return nc.dram_tensor(name, shape, F32, kind="ExternalInput")

    wdve_h = din("wdve", [P, T, 256])
    ident_h = din("ident", [P, P])
    t0_h = din("t0", [P, 48, P])
    t0h_h = din("t0h", [P, 24, P])
    a1_h = din("a1", [P, 24, P])
    a2_h = din("a2", [P, 24, P])
    a3_h = din("a3", [P, 8, P])
    a4_h = din("a4", [P, 32, P])
    a5_h = din("a5", [P, 32, P])
    rnn_h = din("rnn_in_t", [P, 16])
    hp0t_h = din("hp0_t", [P, 8])
    hp1t_h = din("hp1_t", [P, 8])
    hp0s_h = din("hp0_s", [P, 1])
    hp1s_h = din("hp1_s", [P, 1])
    hsel_h = din("hsel", [P, 8])
    b0i_h = din("b0i", [P, 3])
    b0h_h = din("b0h", [P, 3])
    b1i_h = din("b1i", [P, 24])
    b1h_h = din("b1h", [P, 24])
    barr_h = din("barr", [P, T])

    out_logp_h = nc.dram_tensor("out_logp", [P, T], F32, kind="ExternalOutput")
    out_ctx_h = nc.dram_tensor("out_ctx", [P, 1], F32, kind="ExternalOutput")
    out_h0_h = nc.dram_tensor("out_h0", [P, 8], F32, kind="ExternalOutput")
    out_h1_h = nc.dram_tensor("out_h1", [P, 8], F32, kind="ExternalOutput")
    out_attn_h = nc.dram_tensor("out_attn", [P, 32], F32, kind="ExternalOutput")

    with tile.TileContext(nc) as tc:
        with (
            tc.tile_pool(name="sb", bufs=1) as sb,
            tc.tile_pool(name="wp", bufs=7) as wp,
            tc.tile_pool(name="pp", bufs=2) as pp,
            tc.tile_pool(name="ps", bufs=1, space="PSUM") as ps,
            tc.tile_pool(name="psg", bufs=2, space="PSUM") as psg,
            tc.tile_pool(name="dram", bufs=1, space="DRAM") as dram,
        ):
            # ---- warmup collective: absorbs the cold-start latency of the
            # collective engine while the big weight DMAs stream in.
            wz = sb.tile([1, 1], F32, tag="wz")
            nc.vector.memset(wz[:], 0.0)
            warm_in = dram.tile([1, 1], F32, tag="warm_in")
            warm_out = dram.tile([1, 1], F32, tag="warm_out")
            nc.sync.dma_start(warm_in[:], wz[:])
            nc.gpsimd.collective_compute(
                "AllReduce", mybir.AluOpType.add, replica_groups=GROUPS,
                ins=[warm_in.opt()], outs=[warm_out.opt()],
            )
            nc.gpsimd.load_library(library_config.mlp)

            # ---- small resident inputs
            def load(h, shape, tag):
                t_ = sb.tile(shape, F32, tag=tag, name=tag)
                nc.sync.dma_start(t_[:], h[:])
                return t_

            t0_sb = load(t0_h, [P, 48, P], "t0_sb")
            rnn_sb = load(rnn_h, [P, 16], "rnn_sb")
            hp0t_sb = load(hp0t_h, [P, 8], "hp0t_sb")
            hp1t_sb = load(hp1t_h, [P, 8], "hp1t_sb")
            hp0s_sb = load(hp0s_h, [P, 1], "hp0s_sb")
            hp1s_sb = load(hp1s_h, [P, 1], "hp1s_sb")
            hsel_sb = load(hsel_h, [P, 8], "hsel_sb")
            b0i_sb = load(b0i_h, [P, 3], "b0i_sb")
            b0h_sb = load(b0h_h, [P, 3], "b0h_sb")
            b1i_sb = load(b1i_h, [P, 24], "b1i_sb")
            b1h_sb = load(b1h_h, [P, 24], "b1h_sb")
            t0h_sb = load(t0h_h, [P, 24, P], "t0h_sb")
            a1_sb = load(a1_h, [P, 24, P], "a1_sb")
            a2_sb = load(a2_h, [P, 24, P], "a2_sb")
            a3_sb = load(a3_h, [P, 8, P], "a3_sb")
            a4_sb = load(a4_h, [P, 32, P], "a4_sb")
            a5_sb = load(a5_h, [P, 32, P], "a5_sb")
            barr_sb = load(barr_h, [P, T], "barr_sb")
            ident_sb = load(ident_h, [P, P], "ident_sb")

            # ---- GRU layer 0 (output-sharded: this core's 128 units)
            g0i_ps = psg.tile([P, 24], F32, tag="g0", name="g0i_ps")
            g0h_ps = psg.tile([P, 24], F32, tag="g0", name="g0h_ps")
            for g in range(3):
                for kc in range(16):
                    j = kc * 3 + g
                    nc.tensor.matmul(
                        g0i_ps[:, g:g + 1], t0_sb[:, j, :], rnn_sb[:, kc:kc + 1],
                        start=(kc == 0), stop=(kc == 15))
            for g in range(3):
                for kc in range(8):
                    j = kc * 3 + g
                    nc.tensor.matmul(
                        g0h_ps[:, g:g + 1], t0h_sb[:, j, :], hp0t_sb[:, kc:kc + 1],
                        start=(kc == 0), stop=(kc == 7))

            g0i_b = sb.tile([P, 3], F32, tag="g0i_b")
            g0h_b = sb.tile([P, 3], F32, tag="g0h_b")
            nc.vector.tensor_add(g0i_b[:], g0i_ps[:, 0:3], b0i_sb[:])
            nc.vector.tensor_add(g0h_b[:], g0h_ps[:, 0:3], b0h_sb[:])
            r0p = sb.tile([P, 1], F32, tag="r0p")
            nc.vector.tensor_add(r0p[:], g0i_b[:, 0:1], g0h_b[:, 0:1])
            r0 = sb.tile([P, 1], F32, tag="r0")
            nc.scalar.activation(r0[:], r0p[:], AF.Sigmoid)
            z0p = sb.tile([P, 1], F32, tag="z0p")
            nc.vector.tensor_add(z0p[:], g0i_b[:, 1:2], g0h_b[:, 1:2])
            z0 = sb.tile([P, 1], F32, tag="z0")
            nc.scalar.activation(z0[:], z0p[:], AF.Sigmoid)
            n0a = sb.tile([P, 1], F32, tag="n0a")
            nc.vector.tensor_mul(n0a[:], r0[:], g0h_b[:, 2:3])
            n0b = sb.tile([P, 1], F32, tag="n0b")
            nc.vector.tensor_add(n0b[:], n0a[:], g0i_b[:, 2:3])
            n0 = sb.tile([P, 1], F32, tag="n0")
            nc.scalar.activation(n0[:], n0b[:], AF.Tanh)
            d0 = sb.tile([P, 1], F32, tag="d0")
            nc.vector.tensor_sub(d0[:], hp0s_sb[:], n0[:])
            e0 = sb.tile([P, 1], F32, tag="e0")
            nc.vector.tensor_mul(e0[:], z0[:], d0[:])
            h0s = sb.tile([P, 1], F32, tag="h0s")
            nc.vector.tensor_add(h0s[:], n0[:], e0[:])

            # ---- GRU layer 1 partial matmuls (contraction-sharded)
            g1i_ps = psg.tile([P, 24], F32, tag="g0", name="g1i_ps")
            g1h_ps = psg.tile([P, 24], F32, tag="g0", name="g1h_ps")
            for c in range(24):
                nc.tensor.matmul(g1i_ps[:, c:c + 1], a1_sb[:, c, :], h0s[:],
                                 start=True, stop=True)
            for c in range(24):
                nc.tensor.matmul(g1h_ps[:, c:c + 1], a2_sb[:, c, :], hp1s_sb[:],
                                 start=True, stop=True)

            # fused AllReduce payload: [gi1 | gh1 | h0 one-hot-padded]
            ccpay = sb.tile([P, 56], F32, tag="ccpay")
            nc.vector.tensor_copy(ccpay[:, 0:24], g1i_ps[:])
            nc.vector.tensor_copy(ccpay[:, 24:48], g1h_ps[:])
            nc.vector.tensor_scalar_mul(ccpay[:, 48:56], hsel_sb[:], h0s[:])
            cc1_in = dram.tile([P, 56], F32, tag="cc1_in")
            cc1_out = dram.tile([P, 56], F32, tag="cc1_out")
            nc.sync.dma_start(cc1_in[:], ccpay[:])
            nc.gpsimd.collective_compute(
                "AllReduce", mybir.AluOpType.add, replica_groups=GROUPS,
                ins=[cc1_in.opt()], outs=[cc1_out.opt()],
            )
            cc1_sb = sb.tile([P, 56], F32, tag="cc1_sb")
            nc.sync.dma_start(cc1_sb[:], cc1_out[:])
            nc.sync.dma_start(out_h0_h[:], cc1_sb[:, 48:56])

            # ---- GRU layer 1 gates (replicated on every core)
            g1i_b = sb.tile([P, 24], F32, tag="g1i_b")
            g1h_b = sb.tile([P, 24], F32, tag="g1h_b")
            nc.vector.tensor_add(g1i_b[:], cc1_sb[:, 0:24], b1i_sb[:])
            nc.vector.tensor_add(g1h_b[:], cc1_sb[:, 24:48], b1h_sb[:])
            r1p = sb.tile([P, 8], F32, tag="r1p")
            nc.vector.tensor_add(r1p[:], g1i_b[:, 0:8], g1h_b[:, 0:8])
            r1 = sb.tile([P, 8], F32, tag="r1")
            nc.scalar.activation(r1[:], r1p[:], AF.Sigmoid)
            z1p = sb.tile([P, 8], F32, tag="z1p")
            nc.vector.tensor_add(z1p[:], g1i_b[:, 8:16], g1h_b[:, 8:16])
            z1 = sb.tile([P, 8], F32, tag="z1")
            nc.scalar.activation(z1[:], z1p[:], AF.Sigmoid)
            n1a = sb.tile([P, 8], F32, tag="n1a")
            nc.vector.tensor_mul(n1a[:], r1[:], g1h_b[:, 16:24])
            n1b = sb.tile([P, 8], F32, tag="n1b")
            nc.vector.tensor_add(n1b[:], n1a[:], g1i_b[:, 16:24])
            n1 = sb.tile([P, 8], F32, tag="n1")
            nc.scalar.activation(n1[:], n1b[:], AF.Tanh)
            d1 = sb.tile([P, 8], F32, tag="d1")
            nc.vector.tensor_sub(d1[:], hp1t_sb[:], n1[:])
            e1t = sb.tile([P, 8], F32, tag="e1t")
            nc.vector.tensor_mul(e1t[:], z1[:], d1[:])
            h1_sb = sb.tile([P, 8], F32, tag="h1_sb")
            nc.vector.tensor_add(h1_sb[:], n1[:], e1t[:])
            nc.sync.dma_start(out_h1_h[:], h1_sb[:])

            # rhs0[k] = h1[i*128 + k]  (data-driven core-local column select)
            rsel = sb.tile([P, 8], F32, tag="rsel")
            nc.vector.tensor_mul(rsel[:], h1_sb[:], hsel_sb[:])
            rhs0 = sb.tile([P, 1], F32, tag="rhs0")
            nc.vector.reduce_sum(rhs0[:], rsel[:], axis=mybir.AxisListType.X)

            # ---- attention: u shard, partial energies
            u_ps = ps.tile([P, 1], F32, tag="aps", name="u_ps")
            for hc in range(8):
                nc.tensor.matmul(u_ps[:], a3_sb[:, hc, :], h1_sb[:, hc:hc + 1],
                                 start=(hc == 0), stop=(hc == 7))
            u_sb = sb.tile([P, 1], F32, tag="u_sb")
            nc.vector.tensor_copy(u_sb[:], u_ps[:])
            e_ps = ps.tile([P, 32], F32, tag="aps", name="e_ps")
            for t in range(32):
                nc.tensor.matmul(e_ps[:, t:t + 1], a4_sb[:, t, :], u_sb[:],
                                 start=True, stop=True)
            e_in = sb.tile([P, 32], F32, tag="e_in")
            nc.vector.tensor_copy(e_in[:], e_ps[:])
            cc2_in = dram.tile([P, 32], F32, tag="cc2_in")
            cc2_out = dram.tile([P, 32], F32, tag="cc2_out")
            nc.sync.dma_start(cc2_in[:], e_in[:])
            nc.gpsimd.collective_compute(
                "AllReduce", mybir.AluOpType.add, replica_groups=GROUPS,
                ins=[cc2_in.opt()], outs=[cc2_out.opt()],
            )
            e_full = sb.tile([P, 32], F32, tag="e_full")
            nc.sync.dma_start(e_full[:], cc2_out[:])

            # ---- softmax over energies (replicated)
            m1 = sb.tile([P, 1], F32, tag="m1")
            nc.vector.reduce_max(m1[:], e_full[:], axis=mybir.AxisListType.X)
            gm = sb.tile([P, 1], F32, tag="gm")
            nc.gpsimd.partition_all_reduce(gm[:], m1[:], 128, bass_isa.ReduceOp.max)
            ngm = sb.tile([P, 1], F32, tag="ngm")
            nc.vector.tensor_scalar_mul(ngm[:], gm[:], -1.0)
            pexp = sb.tile([P, 32], F32, tag="pexp")
            se = sb.tile([P, 1], F32, tag="se")
            nc.scalar.activation(pexp[:], e_full[:], AF.Exp, bias=ngm[:],
                                 accum_out=se[:])
            gs = sb.tile([P, 1], F32, tag="gs")
            nc.gpsimd.partition_all_reduce(gs[:], se[:], 128, bass_isa.ReduceOp.add)
            ri = sb.tile([P, 1], F32, tag="ri")
            nc.vector.reciprocal(ri[:], gs[:])
            p_sb = sb.tile([P, 32], F32, tag="p_sb")
            nc.vector.tensor_scalar_mul(p_sb[:], pexp[:], ri[:])
            nc.sync.dma_start(out_attn_h[:], p_sb[:])

            # ---- context shard (no collective needed)
            ctx_ps = ps.tile([P, 1], F32, tag="aps", name="ctx_ps")
            for t in range(32):
                nc.tensor.matmul(ctx_ps[:], a5_sb[:, t, :], p_sb[:, t:t + 1],
                                 start=(t == 0), stop=(t == 31))
            ctx_sb = sb.tile([P, 1], F32, tag="ctx_sb")
            nc.vector.tensor_copy(ctx_sb[:], ctx_ps[:])
            nc.sync.dma_start(out_ctx_h[:], ctx_sb[:])

            # ---- x2 = [r_shard | ctx_shard] moved to free-dim layout and
            # broadcast across partitions for the DVE matvec
            rt_ps = psg.tile([1, P], F32, tag="tp", name="rt_ps")
            nc.tensor.transpose(rt_ps[:], rhs0[:], ident_sb[:])
            ct_ps = psg.tile([1, P], F32, tag="tp", name="ct_ps")
            nc.tensor.transpose(ct_ps[:], ctx_sb[:], ident_sb[:])
            x2row = sb.tile([1, 256], F32, tag="x2row")
            nc.vector.tensor_copy(x2row[:, 0:P], rt_ps[:])
            nc.vector.tensor_copy(x2row[:, P:256], ct_ps[:])
            x2bc = sb.tile([P, 1, 256], F32, tag="x2bc")
            nc.gpsimd.partition_broadcast(x2bc[:, 0, :], x2row[:], 128)

            # ---- logits partials: DVE elementwise product per chunk, then
            # per-tile free-dim reduction on the ACT engine (accum_out)
            y_sb = sb.tile([P, T], F32, tag="y_sb")
            ascr = sb.tile([P, 256], F32, tag="ascr")
            t0c = 0
            for ci, tb in enumerate(W_DVE_CHUNKS):
                wt = wp.tile([P, 8, 256], F32, tag="wch", name=f"wt{t0c}")
                nc.sync.dma_start(wt[:, :tb, :], wdve_h[:, t0c:t0c + tb, :])
                prod = pp.tile([P, 8, 256], F32, tag="prod", name=f"prod{t0c}")
                nc.vector.tensor_mul(prod[:, :tb, :], wt[:, :tb, :],
                                     x2bc[:].broadcast_to((P, tb, 256)))
                if ci % 5 < 2:
                    # ACT reduce path: per-tile Identity with free-dim accum
                    for tt in range(tb):
                        t = t0c + tt
                        nc.scalar.activation(ascr[:], prod[:, tt, :], AF.Identity,
                                             accum_out=y_sb[:, t:t + 1])
                else:
                    # DVE reduce path: whole chunk in one op
                    nc.vector.reduce_sum(y_sb[:, t0c:t0c + tb], prod[:, :tb, :],
                                         axis=mybir.AxisListType.X)
                t0c += tb

            cc3_in = dram.tile([P, T], F32, tag="cc3_in")
            cc3_out = dram.tile([P, T], F32, tag="cc3_out")
            nc.sync.dma_start(cc3_in[:], y_sb[:])
            nc.gpsimd.collective_compute(
                "AllReduce", mybir.AluOpType.add, replica_groups=GROUPS,
                ins=[cc3_in.opt()], outs=[cc3_out.opt()],
            )
            cc3_sb = sb.tile([P, T], F32, tag="cc3_sb")
            nc.sync.dma_start(cc3_sb[:], cc3_out[:])

            # ---- log-softmax (replicated)
            lf = sb.tile([P, T], F32, tag="lf")
            nc.vector.tensor_add(lf[:], cc3_sb[:], barr_sb[:])
            m2 = sb.tile([P, 1], F32, tag="m2")
            nc.vector.reduce_max(m2[:], lf[:], axis=mybir.AxisListType.X)
            gm2 = sb.tile([P, 1], F32, tag="gm2")
            nc.gpsimd.partition_all_reduce(gm2[:], m2[:], 128, bass_isa.ReduceOp.max)
            ngm2 = sb.tile([P, 1], F32, tag="ngm2")
            nc.vector.tensor_scalar_mul(ngm2[:], gm2[:], -1.0)
            pex2 = sb.tile([P, T], F32, tag="pex2")
            se2 = sb.tile([P, 1], F32, tag="se2")
            nc.scalar.activation(pex2[:], lf[:], AF.Exp, bias=ngm2[:],
                                 accum_out=se2[:])
            gs2 = sb.tile([P, 1], F32, tag="gs2")
            nc.gpsimd.partition_all_reduce(gs2[:], se2[:], 128, bass_isa.ReduceOp.add)
            l2 = sb.tile([P, 1], F32, tag="l2")
            nc.scalar.activation(l2[:], gs2[:], AF.Ln)
            logz = sb.tile([P, 1], F32, tag="logz")
            nc.vector.tensor_add(logz[:], gm2[:], l2[:])
            outl = sb.tile([P, T], F32, tag="outl")
            nc.vector.tensor_scalar_sub(outl[:], lf[:], logz[:])
            nc.sync.dma_start(out_logp_h[:], outl[:])

    if finalize:
        nc.finalize()
    return nc


def prep_in_maps(inputs):
    """Shard the full (unsharded) problem inputs into 8 per-core input maps."""
    f = np.float32

    def arr(name):
        return np.asarray(inputs[name], f)

    emb_row = np.asarray(inputs["emb"][int(np.asarray(inputs["word_input"]).ravel()[0])], f)
    rnn_in = np.concatenate([emb_row.ravel(), arr("last_context").ravel()])
    rnn_in_t = np.ascontiguousarray(rnn_in.reshape(16, P).T)
    lh = arr("last_hidden")
    hp0, hp1 = lh[0, 0], lh[1, 0]
    hp0_t = np.ascontiguousarray(hp0.reshape(8, P).T)
    hp1_t = np.ascontiguousarray(hp1.reshape(8, P).T)

    t0_all = arr("W_ih0").reshape(3, 8, P, 16, P).transpose(1, 4, 3, 0, 2)
    t0h_all = arr("W_hh0").reshape(3, 8, P, 8, P).transpose(1, 4, 3, 0, 2)
    a1_all = arr("W_ih1").reshape(24, P, 8, P).transpose(2, 3, 0, 1)
    a2_all = arr("W_hh1").reshape(24, P, 8, P).transpose(2, 3, 0, 1)
    a3_all = arr("W_attn").reshape(8, P, 8, P).transpose(2, 1, 0, 3)
    enc = arr("encoder_outputs")[:, 0, :]
    e4 = enc.reshape(32, P, 8, P)
    a4_all = e4.transpose(2, 3, 0, 1)
    a5_all = e4.transpose(2, 1, 0, 3)

    wp_ = np.zeros((V_PAD, 2 * H), f)
    wp_[:V] = arr("W_out")
    # [t, m, kc_all, k] with v = t*128 + m
    wv = wp_.reshape(T, P, 16, P)

    bp = np.full(V_PAD, NEG_BIG, f)
    bp[:V] = arr("b_out")
    barr = np.ascontiguousarray(bp.reshape(T, P).T)
    ident = np.eye(P, dtype=f)

    b0i_all = arr("b_ih0").reshape(3, 8, P).transpose(1, 2, 0)
    b0h_all = arr("b_hh0").reshape(3, 8, P).transpose(1, 2, 0)
    b1i = np.ascontiguousarray(arr("b_ih1").reshape(24, P).T)
    b1h = np.ascontiguousarray(arr("b_hh1").reshape(24, P).T)

    in_maps = []
    for i in range(NC_):
        hsel = np.zeros((P, 8), f)
        hsel[:, i] = 1.0
        wdve = wv[:, :, (i, 8 + i), :].transpose(1, 0, 2, 3).reshape(P, T, 256)
        in_maps.append({
            "wdve": np.ascontiguousarray(wdve),
            "ident": ident,
            "t0": np.ascontiguousarray(t0_all[i].reshape(P, 48, P)),
            "t0h": np.ascontiguousarray(t0h_all[i].reshape(P, 24, P)),
            "a1": np.ascontiguousarray(a1_all[i]),
            "a2": np.ascontiguousarray(a2_all[i]),
            "a3": np.ascontiguousarray(a3_all[i]),
            "a4": np.ascontiguousarray(a4_all[i]),
            "a5": np.ascontiguousarray(a5_all[i]),
            "rnn_in_t": rnn_in_t,
            "hp0_t": hp0_t,
            "hp1_t": hp1_t,
            "hp0_s": np.ascontiguousarray(hp0[i * P:(i + 1) * P].reshape(P, 1)),
            "hp1_s": np.ascontiguousarray(hp1[i * P:(i + 1) * P].reshape(P, 1)),
            "hsel": hsel,
            "b0i": np.ascontiguousarray(b0i_all[i]),
            "b0h": np.ascontiguousarray(b0h_all[i]),
            "b1i": b1i,
            "b1h": b1h,
            "barr": barr,
        })
    return in_maps


def unpack_outputs(results):
    out_logp = np.asarray(results[0]["out_logp"], np.float32).T.ravel()[:V].reshape(1, V)
    ctx = np.concatenate(
        [np.asarray(results[i]["out_ctx"], np.float32).ravel() for i in range(NC_)]
    ).reshape(1, H)
    h0 = np.asarray(results[0]["out_h0"], np.float32).T.ravel()
    h1 = np.asarray(results[0]["out_h1"], np.float32).T.ravel()
    hidden = np.stack([h0, h1]).reshape(2, 1, H)
    attn = np.asarray(results[0]["out_attn"], np.float32).T.ravel().reshape(1, 1, S)
    return out_logp, ctx, hidden, attn


_NC_CACHE = None


def run_on_hw(inputs, trace=False):
    global _NC_CACHE
    if _NC_CACHE is None:
        _NC_CACHE = build_nc()
    in_maps = prep_in_maps(inputs)
    res = run_bass_kernel_spmd(_NC_CACHE, in_maps, list(range(NC_)), trace=trace)
    return unpack_outputs(res.results), res


def kernel(**inputs):
    outs, _ = run_on_hw(inputs, trace=False)
    return outs


# revision 24
# speedup vs baseline: 1.4337x; 1.1565x over previous
"""AttnDecoderRNN single-step decoder on 8 Trainium2 NeuronCores.

Tensor-parallel sharding (hardcoded, 8 cores):
  - GRU layer 0: hidden units sharded (128 per core), full contraction.
  - GRU layer 1: contraction sharded; partial (gi1|gh1|h0) fused AllReduce.
  - Attention: W_attn/encoder columns sharded; partial-energy AllReduce;
    softmax replicated; context column-sharded (no collective).
  - Output projection: contraction (2H) sharded 256/core; partial-logits
    AllReduce; log-softmax replicated locally.
Embedding row gather happens on host (only one row of emb is ever read).
"""

import numpy as np

import concourse.bacc as bacc
import concourse.bass as bass
import concourse.mybir as mybir
import concourse.tile as tile
from concourse import bass_isa
from concourse import library_config
from concourse.bass_utils import run_bass_kernel_spmd

F32 = mybir.dt.float32
NC_ = 8
H = 1024
V = 50257
S = 4096
P = 128
T = 393            # padded vocab tiles: V_pad = 128*393 = 50304
V_PAD = P * T
GROUPS = [list(range(NC_))]
W_CHUNKS = [32] * 12 + [9]   # sum = 393
NEG_BIG = -1.0e30
W_DVE_CHUNKS = [8] * 49 + [1]   # sum = 393

AF = mybir.ActivationFunctionType


def build_nc(finalize=True):
    nc = bacc.Bacc("TRN2", target_bir_lowering=False, debug=False, num_devices=NC_)

    def din(name, shape):
        return nc.dram_tensor(name, shape, F32, kind="ExternalInput")

    wdve_h = din("wdve", [P, T, 256])
    ident_h = din("ident", [P, P])
    t0_h = din("t0", [P, 48, P])
    t0h_h = din("t0h", [P, 24, P])
    a1_h = din("a1", [P, 24, P])
    a2_h = din("a2", [P, 24, P])
    a3_h = din("a3", [P, 8, P])
    a4_h = din("a4", [P, 32, P])
    a5_h = din("a5", [P, 32, P])
    rnn_h = din("rnn_in_t", [P, 16])
    hp0t_h = din("hp0_t", [P, 8])
    hp1t_h = din("hp1_t", [P, 8])
    hp0s_h = din("hp0_s", [P, 1])
    hp1s_h = din("hp1_s", [P, 1])
    hsel_h = din("hsel", [P, 8])
    b0i_h = din("b0i", [P, 3])
    b0h_h = din("b0h", [P, 3])
    b1i_h = din("b1i", [P, 24])
    b1h_h = din("b1h", [P, 24])
    barr_h = din("barr", [P, T])

    out_logp_h = nc.dram_tensor("out_logp", [P, T], F32, kind="ExternalOutput")
    out_ctx_h = nc.dram_tensor("out_ctx", [P, 1], F32, kind="ExternalOutput")
    out_h0_h = nc.dram_tensor("out_h0", [P, 8], F32, kind="ExternalOutput")
    out_h1_h = nc.dram_tensor("out_h1", [P, 8], F32, kind="ExternalOutput")
    out_attn_h = nc.dram_tensor("out_attn", [P, 32], F32, kind="ExternalOutput")

    with tile.TileContext(nc) as tc:
        with (
            tc.tile_pool(name="sb", bufs=1) as sb,
            tc.tile_pool(name="wp", bufs=6) as wp,
            tc.tile_pool(name="pp", bufs=3) as pp,
            tc.tile_pool(name="ps", bufs=1, space="PSUM") as ps,
            tc.tile_pool(name="psg", bufs=2, space="PSUM") as psg,
            tc.tile_pool(name="dram", bufs=1, space="DRAM") as dram,
        ):
            # ---- warmup collective: absorbs the cold-start latency of the
            # collective engine while the big weight DMAs stream in.
            wz = sb.tile([1, 1], F32, tag="wz")
            nc.vector.memset(wz[:], 0.0)
            warm_in = dram.tile([1, 1], F32, tag="warm_in")
            warm_out = dram.tile([1, 1], F32, tag="warm_out")
            nc.sync.dma_start(warm_in[:], wz[:])
            nc.gpsimd.collective_compute(
                "AllReduce", mybir.AluOpType.add, replica_groups=GROUPS,
                ins=[warm_in.opt()], outs=[warm_out.opt()],
            )
            nc.gpsimd.load_library(library_config.mlp)

            # ---- small resident inputs
            def load(h, shape, tag):
                t_ = sb.tile(shape, F32, tag=tag, name=tag)
                nc.sync.dma_start(t_[:], h[:])
                return t_

            t0_sb = load(t0_h, [P, 48, P], "t0_sb")
            rnn_sb = load(rnn_h, [P, 16], "rnn_sb")
            hp0t_sb = load(hp0t_h, [P, 8], "hp0t_sb")
            hp1t_sb = load(hp1t_h, [P, 8], "hp1t_sb")
            hp0s_sb = load(hp0s_h, [P, 1], "hp0s_sb")
            hp1s_sb = load(hp1s_h, [P, 1], "hp1s_sb")
            hsel_sb = load(hsel_h, [P, 8], "hsel_sb")
            b0i_sb = load(b0i_h, [P, 3], "b0i_sb")
            b0h_sb = load(b0h_h, [P, 3], "b0h_sb")
            b1i_sb = load(b1i_h, [P, 24], "b1i_sb")
            b1h_sb = load(b1h_h, [P, 24], "b1h_sb")
            t0h_sb = load(t0h_h, [P, 24, P], "t0h_sb")
            a1_sb = load(a1_h, [P, 24, P], "a1_sb")
            a2_sb = load(a2_h, [P, 24, P], "a2_sb")
            a3_sb = load(a3_h, [P, 8, P], "a3_sb")
            a4_sb = load(a4_h, [P, 32, P], "a4_sb")
            a5_sb = load(a5_h, [P, 32, P], "a5_sb")
            barr_sb = load(barr_h, [P, T], "barr_sb")
            ident_sb = load(ident_h, [P, P], "ident_sb")

            # ---- GRU layer 0 (output-sharded: this core's 128 units)
            g0i_ps = psg.tile([P, 24], F32, tag="g0", name="g0i_ps")
            g0h_ps = psg.tile([P, 24], F32, tag="g0", name="g0h_ps")
            for g in range(3):
                for kc in range(16):
                    j = kc * 3 + g
                    nc.tensor.matmul(
                        g0i_ps[:, g:g + 1], t0_sb[:, j, :], rnn_sb[:, kc:kc + 1],
                        start=(kc == 0), stop=(kc == 15))
            for g in range(3):
                for kc in range(8):
                    j = kc * 3 + g
                    nc.tensor.matmul(
                        g0h_ps[:, g:g + 1], t0h_sb[:, j, :], hp0t_sb[:, kc:kc + 1],
                        start=(kc == 0), stop=(kc == 7))

            g0i_b = sb.tile([P, 3], F32, tag="g0i_b")
            g0h_b = sb.tile([P, 3], F32, tag="g0h_b")
            nc.vector.tensor_add(g0i_b[:], g0i_ps[:, 0:3], b0i_sb[:])
            nc.vector.tensor_add(g0h_b[:], g0h_ps[:, 0:3], b0h_sb[:])
            r0p = sb.tile([P, 1], F32, tag="r0p")
            nc.vector.tensor_add(r0p[:], g0i_b[:, 0:1], g0h_b[:, 0:1])
            r0 = sb.tile([P, 1], F32, tag="r0")
            nc.scalar.activation(r0[:], r0p[:], AF.Sigmoid)
            z0p = sb.tile([P, 1], F32, tag="z0p")
            nc.vector.tensor_add(z0p[:], g0i_b[:, 1:2], g0h_b[:, 1:2])
            z0 = sb.tile([P, 1], F32, tag="z0")
            nc.scalar.activation(z0[:], z0p[:], AF.Sigmoid)
            n0a = sb.tile([P, 1], F32, tag="n0a")
            nc.vector.tensor_mul(n0a[:], r0[:], g0h_b[:, 2:3])
            n0b = sb.tile([P, 1], F32, tag="n0b")
            nc.vector.tensor_add(n0b[:], n0a[:], g0i_b[:, 2:3])
            n0 = sb.tile([P, 1], F32, tag="n0")
            nc.scalar.activation(n0[:], n0b[:], AF.Tanh)
            d0 = sb.tile([P, 1], F32, tag="d0")
            nc.vector.tensor_sub(d0[:], hp0s_sb[:], n0[:])
            e0 = sb.tile([P, 1], F32, tag="e0")
            nc.vector.tensor_mul(e0[:], z0[:], d0[:])
            h0s = sb.tile([P, 1], F32, tag="h0s")
            nc.vector.tensor_add(h0s[:], n0[:], e0[:])

            # ---- GRU layer 1 partial matmuls (contraction-sharded)
            g1i_ps = psg.tile([P, 24], F32, tag="g0", name="g1i_ps")
            g1h_ps = psg.tile([P, 24], F32, tag="g0", name="g1h_ps")
            for c in range(24):
                nc.tensor.matmul(g1i_ps[:, c:c + 1], a1_sb[:, c, :], h0s[:],
                                 start=True, stop=True)
            for c in range(24):
                nc.tensor.matmul(g1h_ps[:, c:c + 1], a2_sb[:, c, :], hp1s_sb[:],
                                 start=True, stop=True)

            # fused AllReduce payload: [gi1 | gh1 | h0 one-hot-padded]
            ccpay = sb.tile([P, 56], F32, tag="ccpay")
            nc.vector.tensor_copy(ccpay[:, 0:24], g1i_ps[:])
            nc.vector.tensor_copy(ccpay[:, 24:48], g1h_ps[:])
            nc.vector.tensor_scalar_mul(ccpay[:, 48:56], hsel_sb[:], h0s[:])
            cc1_in = dram.tile([P, 56], F32, tag="cc1_in")
            cc1_out = dram.tile([P, 56], F32, tag="cc1_out")
            nc.sync.dma_start(cc1_in[:], ccpay[:])
            nc.gpsimd.collective_compute(
                "AllReduce", mybir.AluOpType.add, replica_groups=GROUPS,
                ins=[cc1_in.opt()], outs=[cc1_out.opt()],
            )
            cc1_sb = sb.tile([P, 56], F32, tag="cc1_sb")
            nc.sync.dma_start(cc1_sb[:], cc1_out[:])
            nc.sync.dma_start(out_h0_h[:], cc1_sb[:, 48:56])

            # ---- GRU layer 1 gates (replicated on every core)
            g1i_b = sb.tile([P, 24], F32, tag="g1i_b")
            g1h_b = sb.tile([P, 24], F32, tag="g1h_b")
            nc.vector.tensor_add(g1i_b[:], cc1_sb[:, 0:24], b1i_sb[:])
            nc.vector.tensor_add(g1h_b[:], cc1_sb[:, 24:48], b1h_sb[:])
            r1p = sb.tile([P, 8], F32, tag="r1p")
            nc.vector.tensor_add(r1p[:], g1i_b[:, 0:8], g1h_b[:, 0:8])
            r1 = sb.tile([P, 8], F32, tag="r1")
            nc.scalar.activation(r1[:], r1p[:], AF.Sigmoid)
            z1p = sb.tile([P, 8], F32, tag="z1p")
            nc.vector.tensor_add(z1p[:], g1i_b[:, 8:16], g1h_b[:, 8:16])
            z1 = sb.tile([P, 8], F32, tag="z1")
            nc.scalar.activation(z1[:], z1p[:], AF.Sigmoid)
            n1a = sb.tile([P, 8], F32, tag="n1a")
            nc.vector.tensor_mul(n1a[:], r1[:], g1h_b[:, 16:24])
            n1b = sb.tile([P, 8], F32, tag="n1b")
            nc.vector.tensor_add(n1b[:], n1a[:], g1i_b[:, 16:24])
            n1 = sb.tile([P, 8], F32, tag="n1")
            nc.scalar.activation(n1[:], n1b[:], AF.Tanh)
            d1 = sb.tile([P, 8], F32, tag="d1")
            nc.vector.tensor_sub(d1[:], hp1t_sb[:], n1[:])
            e1t = sb.tile([P, 8], F32, tag="e1t")
            nc.vector.tensor_mul(e1t[:], z1[:], d1[:])
            h1_sb = sb.tile([P, 8], F32, tag="h1_sb")
            nc.vector.tensor_add(h1_sb[:], n1[:], e1t[:])
            nc.sync.dma_start(out_h1_h[:], h1_sb[:])

            # rhs0[k] = h1[i*128 + k]  (data-driven core-local column select)
            rsel = sb.tile([P, 8], F32, tag="rsel")
            nc.vector.tensor_mul(rsel[:], h1_sb[:], hsel_sb[:])
            rhs0 = sb.tile([P, 1], F32, tag="rhs0")
            nc.vector.reduce_sum(rhs0[:], rsel[:], axis=mybir.AxisListType.X)

            # ---- attention: u shard, partial energies
            u_ps = ps.tile([P, 1], F32, tag="aps", name="u_ps")
            for hc in range(8):
                nc.tensor.matmul(u_ps[:], a3_sb[:, hc, :], h1_sb[:, hc:hc + 1],
                                 start=(hc == 0), stop=(hc == 7))
            u_sb = sb.tile([P, 1], F32, tag="u_sb")
            nc.vector.tensor_copy(u_sb[:], u_ps[:])
            e_ps = ps.tile([P, 32], F32, tag="aps", name="e_ps")
            for t in range(32):
                nc.tensor.matmul(e_ps[:, t:t + 1], a4_sb[:, t, :], u_sb[:],
                                 start=True, stop=True)
            e_in = sb.tile([P, 32], F32, tag="e_in")
            nc.vector.tensor_copy(e_in[:], e_ps[:])
            cc2_in = dram.tile([P, 32], F32, tag="cc2_in")
            cc2_out = dram.tile([P, 32], F32, tag="cc2_out")
            nc.sync.dma_start(cc2_in[:], e_in[:])
            nc.gpsimd.collective_compute(
                "AllReduce", mybir.AluOpType.add, replica_groups=GROUPS,
                ins=[cc2_in.opt()], outs=[cc2_out.opt()],
            )
            e_full = sb.tile([P, 32], F32, tag="e_full")
            nc.sync.dma_start(e_full[:], cc2_out[:])

            # ---- softmax over energies (replicated)
            m1 = sb.tile([P, 1], F32, tag="m1")
            nc.vector.reduce_max(m1[:], e_full[:], axis=mybir.AxisListType.X)
            gm = sb.tile([P, 1], F32, tag="gm")
            nc.gpsimd.partition_all_reduce(gm[:], m1[:], 128, bass_isa.ReduceOp.max)
            ngm = sb.tile([P, 1], F32, tag="ngm")
            nc.vector.tensor_scalar_mul(ngm[:], gm[:], -1.0)
            pexp = sb.tile([P, 32], F32, tag="pexp")
            se = sb.tile([P, 1], F32, tag="se")
            nc.scalar.activation(pexp[:], e_full[:], AF.Exp, bias=ngm[:],
                                 accum_out=se[:])
            gs = sb.tile([P, 1], F32, tag="gs")
            nc.gpsimd.partition_all_reduce(gs[:], se[:], 128, bass_isa.ReduceOp.add)
            ri = sb.tile([P, 1], F32, tag="ri")
            nc.vector.reciprocal(ri[:], gs[:])
            p_sb = sb.tile([P, 32], F32, tag="p_sb")
            nc.vector.tensor_scalar_mul(p_sb[:], pexp[:], ri[:])
            nc.sync.dma_start(out_attn_h[:], p_sb[:])

            # ---- context shard (no collective needed)
            ctx_ps = ps.tile([P, 1], F32, tag="aps", name="ctx_ps")
            for t in range(32):
                nc.tensor.matmul(ctx_ps[:], a5_sb[:, t, :], p_sb[:, t:t + 1],
                                 start=(t == 0), stop=(t == 31))
            ctx_sb = sb.tile([P, 1], F32, tag="ctx_sb")
            nc.vector.tensor_copy(ctx_sb[:], ctx_ps[:])
            nc.sync.dma_start(out_ctx_h[:], ctx_sb[:])

            # ---- x2 = [r_shard | ctx_shard] moved to free-dim layout and
            # broadcast across partitions for the DVE matvec
            rt_ps = psg.tile([1, P], F32, tag="tp", name="rt_ps")
            nc.tensor.transpose(rt_ps[:], rhs0[:], ident_sb[:])
            ct_ps = psg.tile([1, P], F32, tag="tp", name="ct_ps")
            nc.tensor.transpose(ct_ps[:], ctx_sb[:], ident_sb[:])
            x2row = sb.tile([1, 256], F32, tag="x2row")
            nc.vector.tensor_copy(x2row[:, 0:P], rt_ps[:])
            nc.vector.tensor_copy(x2row[:, P:256], ct_ps[:])
            x2bc = sb.tile([P, 1, 256], F32, tag="x2bc")
            nc.gpsimd.partition_broadcast(x2bc[:, 0, :], x2row[:], 128)

            # ---- logits partials: DVE elementwise product per chunk, then
            # per-tile free-dim reduction on the ACT engine (accum_out)
            y_sb = sb.tile([P, T], F32, tag="y_sb")
            ascr = sb.tile([P, 256], F32, tag="ascr")
            t0c = 0
            for ci, tb in enumerate(W_DVE_CHUNKS):
                wt = wp.tile([P, 8, 256], F32, tag="wch", name=f"wt{t0c}")
                nc.sync.dma_start(wt[:, :tb, :], wdve_h[:, t0c:t0c + tb, :])
                prod = pp.tile([P, 8, 256], F32, tag="prod", name=f"prod{t0c}")
                nc.vector.tensor_mul(prod[:, :tb, :], wt[:, :tb, :],
                                     x2bc[:].broadcast_to((P, tb, 256)))
                if ci % 5 < 2:
                    # ACT reduce path: per-tile Identity with free-dim accum
                    for tt in range(tb):
                        t = t0c + tt
                        nc.scalar.activation(ascr[:], prod[:, tt, :], AF.Identity,
                                             accum_out=y_sb[:, t:t + 1])
                else:
                    # DVE reduce path: whole chunk in one op
                    nc.vector.reduce_sum(y_sb[:, t0c:t0c + tb], prod[:, :tb, :],
                                         axis=mybir.AxisListType.X)
                t0c += tb

            cc3_in = dram.tile([P, T], F32, tag="cc3_in")
            cc3_out = dram.tile([P, T], F32, tag="cc3_out")
            nc.sync.dma_start(cc3_in[:], y_sb[:])
            nc.gpsimd.collective_compute(
                "AllReduce", mybir.AluOpType.add, replica_groups=GROUPS,
                ins=[cc3_in.opt()], outs=[cc3_out.opt()],
            )
            cc3_sb = sb.tile([P, T], F32, tag="cc3_sb")
            nc.sync.dma_start(cc3_sb[:], cc3_out[:])

            # ---- log-softmax (replicated)
            lf = sb.tile([P, T], F32, tag="lf")
            nc.vector.tensor_add(lf[:], cc3_sb[:], barr_sb[:])
            m2 = sb.tile([P, 1], F32, tag="m2")
            nc.vector.reduce_max(m2[:], lf[:], axis=mybir.AxisListType.X)
            gm2 = sb.tile([P, 1], F32, tag="gm2")
            nc.gpsimd.partition_all_reduce(gm2[:], m2[:], 128, bass_isa.ReduceOp.max)
            ngm2 = sb.tile([P, 1], F32, tag="ngm2")
            nc.vector.tensor_scalar_mul(ngm2[:], gm2[:], -1.0)
            pex2 = sb.tile([P, T], F32, tag="pex2")
            se2 = sb.tile([P, 1], F32, tag="se2")
            nc.scalar.activation(pex2[:], lf[:], AF.Exp, bias=ngm2[:],
                                 accum_out=se2[:])
            gs2 = sb.tile([P, 1], F32, tag="gs2")
            nc.gpsimd.partition_all_reduce(gs2[:], se2[:], 128, bass_isa.ReduceOp.add)
            l2 = sb.tile([P, 1], F32, tag="l2")
            nc.scalar.activation(l2[:], gs2[:], AF.Ln)
            logz = sb.tile([P, 1], F32, tag="logz")
            nc.vector.tensor_add(logz[:], gm2[:], l2[:])
            outl = sb.tile([P, T], F32, tag="outl")
            nc.vector.tensor_scalar_sub(outl[:], lf[:], logz[:])
            nc.sync.dma_start(out_logp_h[:], outl[:])

    if finalize:
        nc.finalize()
    return nc


def prep_in_maps(inputs):
    """Shard the full (unsharded) problem inputs into 8 per-core input maps."""
    f = np.float32

    def arr(name):
        return np.asarray(inputs[name], f)

    emb_row = np.asarray(inputs["emb"][int(np.asarray(inputs["word_input"]).ravel()[0])], f)
    rnn_in = np.concatenate([emb_row.ravel(), arr("last_context").ravel()])
    rnn_in_t = np.ascontiguousarray(rnn_in.reshape(16, P).T)
    lh = arr("last_hidden")
    hp0, hp1 = lh[0, 0], lh[1, 0]
    hp0_t = np.ascontiguousarray(hp0.reshape(8, P).T)
    hp1_t = np.ascontiguousarray(hp1.reshape(8, P).T)

    t0_all = arr("W_ih0").reshape(3, 8, P, 16, P).transpose(1, 4, 3, 0, 2)
    t0h_all = arr("W_hh0").reshape(3, 8, P, 8, P).transpose(1, 4, 3, 0, 2)
    a1_all = arr("W_ih1").reshape(24, P, 8, P).transpose(2, 3, 0, 1)
    a2_all = arr("W_hh1").reshape(24, P, 8, P).transpose(2, 3, 0, 1)
    a3_all = arr("W_attn").reshape(8, P, 8, P).transpose(2, 1, 0, 3)
    enc = arr("encoder_outputs")[:, 0, :]
    e4 = enc.reshape(32, P, 8, P)
    a4_all = e4.transpose(2, 3, 0, 1)
    a5_all = e4.transpose(2, 1, 0, 3)

    wp_ = np.zeros((V_PAD, 2 * H), f)
    wp_[:V] = arr("W_out")
    # [t, m, kc_all, k] with v = t*128 + m
    wv = wp_.reshape(T, P, 16, P)

    bp = np.full(V_PAD, NEG_BIG, f)
    bp[:V] = arr("b_out")
    barr = np.ascontiguousarray(bp.reshape(T, P).T)
    ident = np.eye(P, dtype=f)

    b0i_all = arr("b_ih0").reshape(3, 8, P).transpose(1, 2, 0)
    b0h_all = arr("b_hh0").reshape(3, 8, P).transpose(1, 2, 0)
    b1i = np.ascontiguousarray(arr("b_ih1").reshape(24, P).T)
    b1h = np.ascontiguousarray(arr("b_hh1").reshape(24, P).T)

    in_maps = []
    for i in range(NC_):
        hsel = np.zeros((P, 8), f)
        hsel[:, i] = 1.0
        wdve = wv[:, :, (i, 8 + i), :].transpose(1, 0, 2, 3).reshape(P, T, 256)
        in_maps.append({
            "wdve": np.ascontiguousarray(wdve),
            "ident": ident,
            "t0": np.ascontiguousarray(t0_all[i].reshape(P, 48, P)),
            "t0h": np.ascontiguousarray(t0h_all[i].reshape(P, 24, P)),
            "a1": np.ascontiguousarray(a1_all[i]),
            "a2": np.ascontiguousarray(a2_all[i]),
            "a3": np.ascontiguousarray(a3_all[i]),
            "a4": np.ascontiguousarray(a4_all[i]),
            "a5": np.ascontiguousarray(a5_all[i]),
            "rnn_in_t": rnn_in_t,
            "hp0_t": hp0_t,
            "hp1_t": hp1_t,
            "hp0_s": np.ascontiguousarray(hp0[i * P:(i + 1) * P].reshape(P, 1)),
            "hp1_s": np.ascontiguousarray(hp1[i * P:(i + 1) * P].reshape(P, 1)),
            "hsel": hsel,
            "b0i": np.ascontiguousarray(b0i_all[i]),
            "b0h": np.ascontiguousarray(b0h_all[i]),
            "b1i": b1i,
            "b1h": b1h,
            "barr": barr,
        })
    return in_maps


def unpack_outputs(results):
    out_logp = np.asarray(results[0]["out_logp"], np.float32).T.ravel()[:V].reshape(1, V)
    ctx = np.concatenate(
        [np.asarray(results[i]["out_ctx"], np.float32).ravel() for i in range(NC_)]
    ).reshape(1, H)
    h0 = np.asarray(results[0]["out_h0"], np.float32).T.ravel()
    h1 = np.asarray(results[0]["out_h1"], np.float32).T.ravel()
    hidden = np.stack([h0, h1]).reshape(2, 1, H)
    attn = np.asarray(results[0]["out_attn"], np.float32).T.ravel().reshape(1, 1, S)
    return out_logp, ctx, hidden, attn


_NC_CACHE = None


def run_on_hw(inputs, trace=False):
    global _NC_CACHE
    if _NC_CACHE is None:
        _NC_CACHE = build_nc()
    in_maps = prep_in_maps(inputs)
    res = run_bass_kernel_spmd(_NC_CACHE, in_maps, list(range(NC_)), trace=trace)
    return unpack_outputs(res.results), res


def kernel(**inputs):
    outs, _ = run_on_hw(inputs, trace=False)
    return outs


# revision 25
# speedup vs baseline: 1.4515x; 1.0124x over previous
"""AttnDecoderRNN single-step decoder on 8 Trainium2 NeuronCores.

Tensor-parallel sharding (hardcoded, 8 cores):
  - GRU layer 0: hidden units sharded (128 per core), full contraction.
  - GRU layer 1: contraction sharded; partial (gi1|gh1|h0) fused AllReduce.
  - Attention: W_attn/encoder columns sharded; partial-energy AllReduce;
    softmax replicated; context column-sharded (no collective).
  - Output projection: contraction (2H) sharded 256/core; partial-logits
    AllReduce; log-softmax replicated locally.
Embedding row gather happens on host (only one row of emb is ever read).
"""

import numpy as np

import concourse.bacc as bacc
import concourse.bass as bass
import concourse.mybir as mybir
import concourse.tile as tile
from concourse import bass_isa
from concourse import library_config
from concourse.bass_utils import run_bass_kernel_spmd

F32 = mybir.dt.float32
NC_ = 8
H = 1024
V = 50257
S = 4096
P = 128
T = 393            # padded vocab tiles: V_pad = 128*393 = 50304
V_PAD = P * T
GROUPS = [list(range(NC_))]
W_CHUNKS = [32] * 12 + [9]   # sum = 393
NEG_BIG = -1.0e30
W_DVE_CHUNKS = [8] * 49 + [1]   # sum = 393

AF = mybir.ActivationFunctionType


def build_nc(finalize=True):
    nc = bacc.Bacc("TRN2", target_bir_lowering=False, debug=False, num_devices=NC_)

    def din(name, shape):
        return nc.dram_tensor(name, shape, F32, kind="ExternalInput")

    wdve_h = din("wdve", [P, T, 256])
    ident_h = din("ident", [P, P])
    t0_h = din("t0", [P, 48, P])
    t0h_h = din("t0h", [P, 24, P])
    a1_h = din("a1", [P, 24, P])
    a2_h = din("a2", [P, 24, P])
    a3_h = din("a3", [P, 8, P])
    a4_h = din("a4", [P, 32, P])
    a5_h = din("a5", [P, 32, P])
    rnn_h = din("rnn_in_t", [P, 16])
    hp0t_h = din("hp0_t", [P, 8])
    hp1t_h = din("hp1_t", [P, 8])
    hp0s_h = din("hp0_s", [P, 1])
    hp1s_h = din("hp1_s", [P, 1])
    hsel_h = din("hsel", [P, 8])
    b0i_h = din("b0i", [P, 3])
    b0h_h = din("b0h", [P, 3])
    b1i_h = din("b1i", [P, 24])
    b1h_h = din("b1h", [P, 24])
    barr_h = din("barr", [P, T])

    out_logp_h = nc.dram_tensor("out_logp", [P, T], F32, kind="ExternalOutput")
    out_ctx_h = nc.dram_tensor("out_ctx", [P, 1], F32, kind="ExternalOutput")
    out_h0_h = nc.dram_tensor("out_h0", [P, 8], F32, kind="ExternalOutput")
    out_h1_h = nc.dram_tensor("out_h1", [P, 8], F32, kind="ExternalOutput")
    out_attn_h = nc.dram_tensor("out_attn", [P, 32], F32, kind="ExternalOutput")

    with tile.TileContext(nc) as tc:
        with (
            tc.tile_pool(name="sb", bufs=1) as sb,
            tc.tile_pool(name="wp", bufs=6) as wp,
            tc.tile_pool(name="pp", bufs=3) as pp,
            tc.tile_pool(name="ps", bufs=1, space="PSUM") as ps,
            tc.tile_pool(name="psg", bufs=2, space="PSUM") as psg,
            tc.tile_pool(name="dram", bufs=1, space="DRAM") as dram,
        ):
            # ---- warmup collective: absorbs the cold-start latency of the
            # collective engine while the big weight DMAs stream in.
            wz = sb.tile([1, 1], F32, tag="wz")
            nc.vector.memset(wz[:], 0.0)
            warm_in = dram.tile([1, 1], F32, tag="warm_in")
            warm_out = dram.tile([1, 1], F32, tag="warm_out")
            nc.sync.dma_start(warm_in[:], wz[:])
            nc.gpsimd.collective_compute(
                "AllReduce", mybir.AluOpType.add, replica_groups=GROUPS,
                ins=[warm_in.opt()], outs=[warm_out.opt()],
            )
            nc.gpsimd.load_library(library_config.mlp)

            # ---- small resident inputs
            def load(h, shape, tag):
                t_ = sb.tile(shape, F32, tag=tag, name=tag)
                nc.sync.dma_start(t_[:], h[:])
                return t_

            t0_sb = load(t0_h, [P, 48, P], "t0_sb")
            rnn_sb = load(rnn_h, [P, 16], "rnn_sb")
            hp0t_sb = load(hp0t_h, [P, 8], "hp0t_sb")
            hp1t_sb = load(hp1t_h, [P, 8], "hp1t_sb")
            hp0s_sb = load(hp0s_h, [P, 1], "hp0s_sb")
            hp1s_sb = load(hp1s_h, [P, 1], "hp1s_sb")
            hsel_sb = load(hsel_h, [P, 8], "hsel_sb")
            b0i_sb = load(b0i_h, [P, 3], "b0i_sb")
            b0h_sb = load(b0h_h, [P, 3], "b0h_sb")
            b1i_sb = load(b1i_h, [P, 24], "b1i_sb")
            b1h_sb = load(b1h_h, [P, 24], "b1h_sb")
            t0h_sb = load(t0h_h, [P, 24, P], "t0h_sb")
            a1_sb = load(a1_h, [P, 24, P], "a1_sb")
            a2_sb = load(a2_h, [P, 24, P], "a2_sb")
            a3_sb = load(a3_h, [P, 8, P], "a3_sb")
            a4_sb = load(a4_h, [P, 32, P], "a4_sb")
            a5_sb = load(a5_h, [P, 32, P], "a5_sb")
            barr_sb = load(barr_h, [P, T], "barr_sb")
            ident_sb = load(ident_h, [P, P], "ident_sb")

            # ---- GRU layer 0 (output-sharded: this core's 128 units)
            g0i_ps = psg.tile([P, 24], F32, tag="g0", name="g0i_ps")
            g0h_ps = psg.tile([P, 24], F32, tag="g0", name="g0h_ps")
            for g in range(3):
                for kc in range(16):
                    j = kc * 3 + g
                    nc.tensor.matmul(
                        g0i_ps[:, g:g + 1], t0_sb[:, j, :], rnn_sb[:, kc:kc + 1],
                        start=(kc == 0), stop=(kc == 15))
            for g in range(3):
                for kc in range(8):
                    j = kc * 3 + g
                    nc.tensor.matmul(
                        g0h_ps[:, g:g + 1], t0h_sb[:, j, :], hp0t_sb[:, kc:kc + 1],
                        start=(kc == 0), stop=(kc == 7))

            g0i_b = sb.tile([P, 3], F32, tag="g0i_b")
            g0h_b = sb.tile([P, 3], F32, tag="g0h_b")
            nc.vector.tensor_add(g0i_b[:], g0i_ps[:, 0:3], b0i_sb[:])
            nc.vector.tensor_add(g0h_b[:], g0h_ps[:, 0:3], b0h_sb[:])
            r0p = sb.tile([P, 1], F32, tag="r0p")
            nc.vector.tensor_add(r0p[:], g0i_b[:, 0:1], g0h_b[:, 0:1])
            r0 = sb.tile([P, 1], F32, tag="r0")
            nc.scalar.activation(r0[:], r0p[:], AF.Sigmoid)
            z0p = sb.tile([P, 1], F32, tag="z0p")
            nc.vector.tensor_add(z0p[:], g0i_b[:, 1:2], g0h_b[:, 1:2])
            z0 = sb.tile([P, 1], F32, tag="z0")
            nc.scalar.activation(z0[:], z0p[:], AF.Sigmoid)
            n0a = sb.tile([P, 1], F32, tag="n0a")
            nc.vector.tensor_mul(n0a[:], r0[:], g0h_b[:, 2:3])
            n0b = sb.tile([P, 1], F32, tag="n0b")
            nc.vector.tensor_add(n0b[:], n0a[:], g0i_b[:, 2:3])
            n0 = sb.tile([P, 1], F32, tag="n0")
            nc.scalar.activation(n0[:], n0b[:], AF.Tanh)
            d0 = sb.tile([P, 1], F32, tag="d0")
            nc.vector.tensor_sub(d0[:], hp0s_sb[:], n0[:])
            e0 = sb.tile([P, 1], F32, tag="e0")
            nc.vector.tensor_mul(e0[:], z0[:], d0[:])
            h0s = sb.tile([P, 1], F32, tag="h0s")
            nc.vector.tensor_add(h0s[:], n0[:], e0[:])

            # ---- GRU layer 1 partial matmuls (contraction-sharded)
            g1i_ps = psg.tile([P, 24], F32, tag="g0", name="g1i_ps")
            g1h_ps = psg.tile([P, 24], F32, tag="g0", name="g1h_ps")
            for c in range(24):
                nc.tensor.matmul(g1i_ps[:, c:c + 1], a1_sb[:, c, :], h0s[:],
                                 start=True, stop=True)
            for c in range(24):
                nc.tensor.matmul(g1h_ps[:, c:c + 1], a2_sb[:, c, :], hp1s_sb[:],
                                 start=True, stop=True)

            # fused AllReduce payload: [gi1 | gh1 | h0 one-hot-padded]
            ccpay = sb.tile([P, 56], F32, tag="ccpay")
            nc.vector.tensor_copy(ccpay[:, 0:24], g1i_ps[:])
            nc.vector.tensor_copy(ccpay[:, 24:48], g1h_ps[:])
            nc.vector.tensor_scalar_mul(ccpay[:, 48:56], hsel_sb[:], h0s[:])
            cc1_in = dram.tile([P, 56], F32, tag="cc1_in")
            cc1_out = dram.tile([P, 56], F32, tag="cc1_out")
            nc.sync.dma_start(cc1_in[:], ccpay[:])
            nc.gpsimd.collective_compute(
                "AllReduce", mybir.AluOpType.add, replica_groups=GROUPS,
                ins=[cc1_in.opt()], outs=[cc1_out.opt()],
            )
            cc1_sb = sb.tile([P, 56], F32, tag="cc1_sb")
            nc.sync.dma_start(cc1_sb[:], cc1_out[:])
            nc.sync.dma_start(out_h0_h[:], cc1_sb[:, 48:56])

            # ---- GRU layer 1 gates (replicated on every core)
            g1i_b = sb.tile([P, 24], F32, tag="g1i_b")
            g1h_b = sb.tile([P, 24], F32, tag="g1h_b")
            nc.vector.tensor_add(g1i_b[:], cc1_sb[:, 0:24], b1i_sb[:])
            nc.vector.tensor_add(g1h_b[:], cc1_sb[:, 24:48], b1h_sb[:])
            r1p = sb.tile([P, 8], F32, tag="r1p")
            nc.vector.tensor_add(r1p[:], g1i_b[:, 0:8], g1h_b[:, 0:8])
            r1 = sb.tile([P, 8], F32, tag="r1")
            nc.scalar.activation(r1[:], r1p[:], AF.Sigmoid)
            z1p = sb.tile([P, 8], F32, tag="z1p")
            nc.vector.tensor_add(z1p[:], g1i_b[:, 8:16], g1h_b[:, 8:16])
            z1 = sb.tile([P, 8], F32, tag="z1")
            nc.scalar.activation(z1[:], z1p[:], AF.Sigmoid)
            n1a = sb.tile([P, 8], F32, tag="n1a")
            nc.vector.tensor_mul(n1a[:], r1[:], g1h_b[:, 16:24])
            n1b = sb.tile([P, 8], F32, tag="n1b")
            nc.vector.tensor_add(n1b[:], n1a[:], g1i_b[:, 16:24])
            n1 = sb.tile([P, 8], F32, tag="n1")
            nc.scalar.activation(n1[:], n1b[:], AF.Tanh)
            d1 = sb.tile([P, 8], F32, tag="d1")
            nc.vector.tensor_sub(d1[:], hp1t_sb[:], n1[:])
            e1t = sb.tile([P, 8], F32, tag="e1t")
            nc.vector.tensor_mul(e1t[:], z1[:], d1[:])
            h1_sb = sb.tile([P, 8], F32, tag="h1_sb")
            nc.vector.tensor_add(h1_sb[:], n1[:], e1t[:])
            nc.sync.dma_start(out_h1_h[:], h1_sb[:])

            # rhs0[k] = h1[i*128 + k]  (data-driven core-local column select)
            rsel = sb.tile([P, 8], F32, tag="rsel")
            nc.vector.tensor_mul(rsel[:], h1_sb[:], hsel_sb[:])
            rhs0 = sb.tile([P, 1], F32, tag="rhs0")
            nc.vector.reduce_sum(rhs0[:], rsel[:], axis=mybir.AxisListType.X)

            # ---- attention: u shard, partial energies
            u_ps = ps.tile([P, 1], F32, tag="aps", name="u_ps")
            for hc in range(8):
                nc.tensor.matmul(u_ps[:], a3_sb[:, hc, :], h1_sb[:, hc:hc + 1],
                                 start=(hc == 0), stop=(hc == 7))
            u_sb = sb.tile([P, 1], F32, tag="u_sb")
            nc.vector.tensor_copy(u_sb[:], u_ps[:])
            e_ps = ps.tile([P, 32], F32, tag="aps", name="e_ps")
            for t in range(32):
                nc.tensor.matmul(e_ps[:, t:t + 1], a4_sb[:, t, :], u_sb[:],
                                 start=True, stop=True)
            e_in = sb.tile([P, 32], F32, tag="e_in")
            nc.vector.tensor_copy(e_in[:], e_ps[:])
            cc2_in = dram.tile([P, 32], F32, tag="cc2_in")
            cc2_out = dram.tile([P, 32], F32, tag="cc2_out")
            nc.sync.dma_start(cc2_in[:], e_in[:])
            nc.gpsimd.collective_compute(
                "AllReduce", mybir.AluOpType.add, replica_groups=GROUPS,
                ins=[cc2_in.opt()], outs=[cc2_out.opt()],
            )
            e_full = sb.tile([P, 32], F32, tag="e_full")
            nc.sync.dma_start(e_full[:], cc2_out[:])

            # ---- softmax over energies (replicated)
            m1 = sb.tile([P, 1], F32, tag="m1")
            nc.vector.reduce_max(m1[:], e_full[:], axis=mybir.AxisListType.X)
            gm = sb.tile([P, 1], F32, tag="gm")
            nc.gpsimd.partition_all_reduce(gm[:], m1[:], 128, bass_isa.ReduceOp.max)
            ngm = sb.tile([P, 1], F32, tag="ngm")
            nc.vector.tensor_scalar_mul(ngm[:], gm[:], -1.0)
            pexp = sb.tile([P, 32], F32, tag="pexp")
            se = sb.tile([P, 1], F32, tag="se")
            nc.scalar.activation(pexp[:], e_full[:], AF.Exp, bias=ngm[:],
                                 accum_out=se[:])
            gs = sb.tile([P, 1], F32, tag="gs")
            nc.gpsimd.partition_all_reduce(gs[:], se[:], 128, bass_isa.ReduceOp.add)
            ri = sb.tile([P, 1], F32, tag="ri")
            nc.vector.reciprocal(ri[:], gs[:])
            p_sb = sb.tile([P, 32], F32, tag="p_sb")
            nc.vector.tensor_scalar_mul(p_sb[:], pexp[:], ri[:])
            nc.sync.dma_start(out_attn_h[:], p_sb[:])

            # ---- context shard (no collective needed)
            ctx_ps = ps.tile([P, 1], F32, tag="aps", name="ctx_ps")
            for t in range(32):
                nc.tensor.matmul(ctx_ps[:], a5_sb[:, t, :], p_sb[:, t:t + 1],
                                 start=(t == 0), stop=(t == 31))
            ctx_sb = sb.tile([P, 1], F32, tag="ctx_sb")
            nc.vector.tensor_copy(ctx_sb[:], ctx_ps[:])
            nc.sync.dma_start(out_ctx_h[:], ctx_sb[:])

            # ---- x2 = [r_shard | ctx_shard] moved to free-dim layout and
            # broadcast across partitions for the DVE matvec
            rt_ps = psg.tile([1, P], F32, tag="tp", name="rt_ps")
            nc.tensor.transpose(rt_ps[:], rhs0[:], ident_sb[:])
            ct_ps = psg.tile([1, P], F32, tag="tp", name="ct_ps")
            nc.tensor.transpose(ct_ps[:], ctx_sb[:], ident_sb[:])
            x2row = sb.tile([1, 256], F32, tag="x2row")
            nc.vector.tensor_copy(x2row[:, 0:P], rt_ps[:])
            nc.vector.tensor_copy(x2row[:, P:256], ct_ps[:])
            x2bc = sb.tile([P, 1, 256], F32, tag="x2bc")
            nc.gpsimd.partition_broadcast(x2bc[:, 0, :], x2row[:], 128)

            # ---- logits partials: DVE elementwise product per chunk, then
            # per-tile free-dim reduction on the ACT engine (accum_out)
            y_sb = sb.tile([P, T], F32, tag="y_sb")
            ascr = sb.tile([P, 256], F32, tag="ascr")
            t0c = 0
            for ci, tb in enumerate(W_DVE_CHUNKS):
                wt = wp.tile([P, 8, 256], F32, tag="wch", name=f"wt{t0c}")
                nc.sync.dma_start(wt[:, :tb, :], wdve_h[:, t0c:t0c + tb, :])
                prod = pp.tile([P, 8, 256], F32, tag="prod", name=f"prod{t0c}")
                nc.vector.tensor_mul(prod[:, :tb, :], wt[:, :tb, :],
                                     x2bc[:].broadcast_to((P, tb, 256)))
                if ci % 2 == 0:
                    # ACT reduce path: per-tile Identity with free-dim accum
                    for tt in range(tb):
                        t = t0c + tt
                        nc.scalar.activation(ascr[:], prod[:, tt, :], AF.Identity,
                                             accum_out=y_sb[:, t:t + 1])
                else:
                    # DVE reduce path: whole chunk in one op
                    nc.vector.reduce_sum(y_sb[:, t0c:t0c + tb], prod[:, :tb, :],
                                         axis=mybir.AxisListType.X)
                t0c += tb

            cc3_in = dram.tile([P, T], F32, tag="cc3_in")
            cc3_out = dram.tile([P, T], F32, tag="cc3_out")
            nc.sync.dma_start(cc3_in[:], y_sb[:])
            nc.gpsimd.collective_compute(
                "AllReduce", mybir.AluOpType.add, replica_groups=GROUPS,
                ins=[cc3_in.opt()], outs=[cc3_out.opt()],
            )
            cc3_sb = sb.tile([P, T], F32, tag="cc3_sb")
            nc.sync.dma_start(cc3_sb[:], cc3_out[:])

            # ---- log-softmax (replicated)
            lf = sb.tile([P, T], F32, tag="lf")
            nc.vector.tensor_add(lf[:], cc3_sb[:], barr_sb[:])
            m2 = sb.tile([P, 1], F32, tag="m2")
            nc.vector.reduce_max(m2[:], lf[:], axis=mybir.AxisListType.X)
            gm2 = sb.tile([P, 1], F32, tag="gm2")
            nc.gpsimd.partition_all_reduce(gm2[:], m2[:], 128, bass_isa.ReduceOp.max)
            ngm2 = sb.tile([P, 1], F32, tag="ngm2")
            nc.vector.tensor_scalar_mul(ngm2[:], gm2[:], -1.0)
            pex2 = sb.tile([P, T], F32, tag="pex2")
            se2 = sb.tile([P, 1], F32, tag="se2")
            nc.scalar.activation(pex2[:], lf[:], AF.Exp, bias=ngm2[:],
                                 accum_out=se2[:])
            gs2 = sb.tile([P, 1], F32, tag="gs2")
            nc.gpsimd.partition_all_reduce(gs2[:], se2[:], 128, bass_isa.ReduceOp.add)
            l2 = sb.tile([P, 1], F32, tag="l2")
            nc.scalar.activation(l2[:], gs2[:], AF.Ln)
            logz = sb.tile([P, 1], F32, tag="logz")
            nc.vector.tensor_add(logz[:], gm2[:], l2[:])
            outl = sb.tile([P, T], F32, tag="outl")
            nc.vector.tensor_scalar_sub(outl[:], lf[:], logz[:])
            nc.sync.dma_start(out_logp_h[:], outl[:])

    if finalize:
        nc.finalize()
    return nc


def prep_in_maps(inputs):
    """Shard the full (unsharded) problem inputs into 8 per-core input maps."""
    f = np.float32

    def arr(name):
        return np.asarray(inputs[name], f)

    emb_row = np.asarray(inputs["emb"][int(np.asarray(inputs["word_input"]).ravel()[0])], f)
    rnn_in = np.concatenate([emb_row.ravel(), arr("last_context").ravel()])
    rnn_in_t = np.ascontiguousarray(rnn_in.reshape(16, P).T)
    lh = arr("last_hidden")
    hp0, hp1 = lh[0, 0], lh[1, 0]
    hp0_t = np.ascontiguousarray(hp0.reshape(8, P).T)
    hp1_t = np.ascontiguousarray(hp1.reshape(8, P).T)

    t0_all = arr("W_ih0").reshape(3, 8, P, 16, P).transpose(1, 4, 3, 0, 2)
    t0h_all = arr("W_hh0").reshape(3, 8, P, 8, P).transpose(1, 4, 3, 0, 2)
    a1_all = arr("W_ih1").reshape(24, P, 8, P).transpose(2, 3, 0, 1)
    a2_all = arr("W_hh1").reshape(24, P, 8, P).transpose(2, 3, 0, 1)
    a3_all = arr("W_attn").reshape(8, P, 8, P).transpose(2, 1, 0, 3)
    enc = arr("encoder_outputs")[:, 0, :]
    e4 = enc.reshape(32, P, 8, P)
    a4_all = e4.transpose(2, 3, 0, 1)
    a5_all = e4.transpose(2, 1, 0, 3)

    wp_ = np.zeros((V_PAD, 2 * H), f)
    wp_[:V] = arr("W_out")
    # [t, m, kc_all, k] with v = t*128 + m
    wv = wp_.reshape(T, P, 16, P)

    bp = np.full(V_PAD, NEG_BIG, f)
    bp[:V] = arr("b_out")
    barr = np.ascontiguousarray(bp.reshape(T, P).T)
    ident = np.eye(P, dtype=f)

    b0i_all = arr("b_ih0").reshape(3, 8, P).transpose(1, 2, 0)
    b0h_all = arr("b_hh0").reshape(3, 8, P).transpose(1, 2, 0)
    b1i = np.ascontiguousarray(arr("b_ih1").reshape(24, P).T)
    b1h = np.ascontiguousarray(arr("b_hh1").reshape(24, P).T)

    in_maps = []
    for i in range(NC_):
        hsel = np.zeros((P, 8), f)
        hsel[:, i] = 1.0
        wdve = wv[:, :, (i, 8 + i), :].transpose(1, 0, 2, 3).reshape(P, T, 256)
        in_maps.append({
            "wdve": np.ascontiguousarray(wdve),
            "ident": ident,
            "t0": np.ascontiguousarray(t0_all[i].reshape(P, 48, P)),
            "t0h": np.ascontiguousarray(t0h_all[i].reshape(P, 24, P)),
            "a1": np.ascontiguousarray(a1_all[i]),
            "a2": np.ascontiguousarray(a2_all[i]),
            "a3": np.ascontiguousarray(a3_all[i]),
            "a4": np.ascontiguousarray(a4_all[i]),
            "a5": np.ascontiguousarray(a5_all[i]),
            "rnn_in_t": rnn_in_t,
            "hp0_t": hp0_t,
            "hp1_t": hp1_t,
            "hp0_s": np.ascontiguousarray(hp0[i * P:(i + 1) * P].reshape(P, 1)),
            "hp1_s": np.ascontiguousarray(hp1[i * P:(i + 1) * P].reshape(P, 1)),
            "hsel": hsel,
            "b0i": np.ascontiguousarray(b0i_all[i]),
            "b0h": np.ascontiguousarray(b0h_all[i]),
            "b1i": b1i,
            "b1h": b1h,
            "barr": barr,
        })
    return in_maps


def unpack_outputs(results):
    out_logp = np.asarray(results[0]["out_logp"], np.float32).T.ravel()[:V].reshape(1, V)
    ctx = np.concatenate(
        [np.asarray(results[i]["out_ctx"], np.float32).ravel() for i in range(NC_)]
    ).reshape(1, H)
    h0 = np.asarray(results[0]["out_h0"], np.float32).T.ravel()
    h1 = np.asarray(results[0]["out_h1"], np.float32).T.ravel()
    hidden = np.stack([h0, h1]).reshape(2, 1, H)
    attn = np.asarray(results[0]["out_attn"], np.float32).T.ravel().reshape(1, 1, S)
    return out_logp, ctx, hidden, attn


_NC_CACHE = None


def run_on_hw(inputs, trace=False):
    global _NC_CACHE
    if _NC_CACHE is None:
        _NC_CACHE = build_nc()
    in_maps = prep_in_maps(inputs)
    res = run_bass_kernel_spmd(_NC_CACHE, in_maps, list(range(NC_)), trace=trace)
    return unpack_outputs(res.results), res


def kernel(**inputs):
    outs, _ = run_on_hw(inputs, trace=False)
    return outs
